# revision 1
# baseline (speedup 1.0000x reference)
"""CuGraphSAGEConv (mean aggregation) on 8 trn2 NeuronCores via raw Bass.

Strategy
--------
Target nodes are sharded across 8 cores (edge-balanced CSC split); x is
replicated. Per core, each node's degree is decomposed exactly into parts
from a fixed class set (no wasted gather descriptors). Per (band, class):
one or two indirect-DMA gathers pull x[row[e]] rows into SBUF laid out as
[partition, group, K, 64]; a log-depth DVE fold reduces each K-group to a
partial node sum; partials are compacted and staged to DRAM (S1).
Multi-part nodes get a small second gather+fold ("combine") whose result
is written back to S1. A final gather collects one S1 row per node in
natural order; scale by 1/deg, PE-transpose, one matmul with W^T
(128->64), bias, transpose back, contiguous store.

The walrus build in this environment only accepts ONE attached sync-wait
per instruction, so everything is raw Bass with standalone wait_ge's.
Indirect-DMA facts established by probing the hardware:
  - idx tile [P, M] gathers row idx[p, m] into out[p, m*64:(m+1)*64]
    (p-major flat order); dest may be a partition/column slice.
  - out-of-bounds skipping only works for a tail suffix, so all padding
    indices here are in-bounds (row 0) and their results are discarded.
  - compute_op=add accumulates exactly.
"""

import os
import numpy as np

N = 100_000
E = 1_600_000
D = 64
NCORES = 8
P = 128

# exact degree decomposition classes (descending)
CLASSES = list(range(64, 32, -4)) + list(range(32, 0, -1))
SLOTS = 128          # gather-slot capacity per partition per chunk
GBUFS = 2            # chunk gather tile buffers
IBUFS = 3            # idx tile buffers
SPLIT_PIECE_MIN_K = 13  # classes >= this get a separate remainder gather

LAST_EXEC_TIME_NS = None
LAST_MEAN_EXEC_TIME_NS = None


def _decompose(d):
    parts = []
    while d > 64:
        parts.append(64)
        d -= 64
    if d > 32:
        b = d - (d % 4) if d >= 36 else 32
        parts.append(b)
        d -= b
    if d > 0:
        parts.append(d)
    return parts


def _fold_steps(w):
    steps = []
    while w > 1:
        h = 1 << ((w - 1).bit_length() - 1)
        if h == w:
            h = w // 2
        steps.append((h, w))
        w = h
    return steps


def _plan_and_arrays(x, row, colptr, W, b):
    deg = np.diff(colptr).astype(np.int64)

    cuts = np.searchsorted(colptr, (np.arange(1, NCORES) * (E // NCORES)).astype(colptr.dtype))
    bounds = np.concatenate([[0], cuts, [N]]).astype(np.int64)
    counts = np.diff(bounds)
    maxcount = int(counts.max())

    TB = (maxcount + 255) // 256
    band_nodes = TB * P
    ncap = 2 * band_nodes

    core_members = []
    core_nparts = []
    for c in range(NCORES):
        n0, n1 = int(bounds[c]), int(bounds[c + 1])
        cnt = n1 - n0
        dg = deg[n0:n1]
        npart = np.zeros(max(cnt, 1), np.int64)
        bands = [dict(), dict()]
        for loc in range(cnt):
            d = int(dg[loc])
            if d == 0:
                continue
            parts = _decompose(d)
            npart[loc] = len(parts)
            beta = 0 if loc < band_nodes else 1
            es = int(colptr[n0 + loc])
            for k, K in enumerate(parts):
                bands[beta].setdefault(K, []).append((loc, es, k))
                es += K
        core_members.append(bands)
        core_nparts.append(npart)

    PMAX = max(2, max(int(cn.max()) for cn in core_nparts))

    nmax = [dict(), dict()]
    for beta in range(2):
        for K in CLASSES:
            m = max(len(core_members[c][beta].get(K, [])) for c in range(NCORES))
            if m:
                nmax[beta][K] = m

    ncmb_max = [0, 0]
    for beta in range(2):
        for c in range(NCORES):
            npc = core_nparts[c]
            lo = beta * band_nodes
            hi = min(int(counts[c]), (beta + 1) * band_nodes)
            if hi > lo:
                ncmb_max[beta] = max(ncmb_max[beta], int((npc[lo:hi] >= 2).sum()))
    CM = [((ncmb_max[b] + P - 1) // P) if ncmb_max[b] else 0 for b in range(2)]

    band_layout = []
    for beta in range(2):
        pieces = []
        for K in CLASSES:
            n = nmax[beta].get(K, 0)
            if n == 0:
                continue
            if K >= SPLIT_PIECE_MIN_K:
                G = n // P
                r = n % P
                off = 0
                while G > 0:
                    g = min(G, SLOTS // K)
                    pieces.append(dict(K=K, ph=P, G=g, kind="bulk", mem_off=off))
                    off += P * g
                    G -= g
                if r:
                    pieces.append(dict(K=K, ph=r, G=1, kind="rem", mem_off=off))
            else:
                G1 = (n + P - 1) // P
                off = 0
                while G1 > 0:
                    g = min(G1, SLOTS // K)
                    pieces.append(dict(K=K, ph=P, G=g, kind="pad", mem_off=off))
                    off += P * g
                    G1 -= g
        chunk_cols = []
        for pc in pieces:
            w = pc["G"] * pc["K"]
            for ci, used in enumerate(chunk_cols):
                if used + w <= SLOTS:
                    pc["chunk"] = ci
                    pc["col0"] = used
                    chunk_cols[ci] = used + w
                    break
            else:
                pc["chunk"] = len(chunk_cols)
                pc["col0"] = 0
                chunk_cols.append(w)
        NC = len(chunk_cols)
        s1c = 1
        for pc in pieces:
            pc["s1c0"] = s1c
            s1c += pc["G"]
        cmb_col0 = s1c
        R = s1c + CM[beta]
        band_layout.append(dict(pieces=pieces, NC=NC, R=R, cmb_col0=cmb_col0))

    NC_tot = band_layout[0]["NC"] + band_layout[1]["NC"]

    row32 = row.astype(np.int32, copy=False)
    in_maps = []
    for c in range(NCORES):
        n0 = int(bounds[c])
        cnt = int(counts[c])
        idxf = np.zeros((NC_tot, P, SLOTS), np.int32)
        parts_rows = np.zeros((ncap, PMAX), np.int32)
        parts_cnt = np.zeros(ncap, np.int32)

        for beta in range(2):
            lay = band_layout[beta]
            R = lay["R"]
            chunk_base = 0 if beta == 0 else band_layout[0]["NC"]
            for K in CLASSES:
                if K not in nmax[beta]:
                    continue
                mem = core_members[c][beta].get(K, [])
                n = len(mem)
                if n:
                    nodes = np.array([m[0] for m in mem], np.int64)
                    es = np.array([m[1] for m in mem], np.int64)
                    ords = np.array([m[2] for m in mem], np.int64)
                    vals = row32[(es[:, None] + np.arange(K)).reshape(-1)].reshape(n, K)
                for pc in lay["pieces"]:
                    if pc["K"] != K:
                        continue
                    o = pc["mem_off"]
                    ncells = pc["ph"] * pc["G"]
                    a = np.zeros((ncells, K), np.int32)
                    take = max(0, min(n - o, ncells)) if n else 0
                    if take:
                        a[:take] = vals[o:o + take]
                    ci = chunk_base + pc["chunk"]
                    s0 = pc["col0"]
                    if pc["kind"] == "rem":
                        idxf[ci, :pc["ph"], s0:s0 + K] = a
                    else:
                        idxf[ci, :, s0:s0 + pc["G"] * K] = a.reshape(P, pc["G"] * K)
                    if take:
                        j = np.arange(take)
                        if pc["kind"] == "rem":
                            rows = j * R + pc["s1c0"]
                        else:
                            rows = (j // pc["G"]) * R + pc["s1c0"] + (j % pc["G"])
                        nd = nodes[o:o + take]
                        od = ords[o:o + take]
                        parts_rows[nd, od] = rows.astype(np.int32)
                        parts_cnt[nd] = np.maximum(parts_cnt[nd], (od + 1).astype(np.int32))

        cmbi = []
        l2i = np.zeros((2, P, TB), np.int32)
        for beta in range(2):
            lay = band_layout[beta]
            R = lay["R"]
            cmw = max(1, CM[beta] * PMAX)
            ci_arr = np.zeros((P, cmw), np.int32)
            lo = beta * band_nodes
            hi = min(cnt, (beta + 1) * band_nodes)
            node_row = np.zeros(band_nodes, np.int32)
            if hi > lo:
                loc = np.arange(lo, hi)
                pcn = parts_cnt[lo:hi]
                single = pcn == 1
                node_row[:hi - lo][single] = parts_rows[lo:hi][single, 0]
                multi_nodes = loc[pcn >= 2]
                for m, nd in enumerate(multi_nodes):
                    p_ = m // CM[beta]
                    jm = m % CM[beta]
                    for k in range(int(parts_cnt[nd])):
                        ci_arr[p_, jm * PMAX + k] = parts_rows[nd, k]
                    node_row[nd - lo] = p_ * R + lay["cmb_col0"] + jm
            cmbi.append(ci_arr)
            l2i[beta] = node_row.reshape(TB, P).T

        recip = np.ones(ncap, np.float32)
        recip[:cnt] = 1.0 / np.maximum(deg[n0:n0 + cnt].astype(np.float32), 1.0)
        recip_pk = recip.reshape(2, TB, P).transpose(0, 2, 1).copy()
        xroot = np.zeros((ncap, D), np.float32)
        xroot[:cnt] = x[n0:n0 + cnt]

        in_maps.append({
            "x": np.ascontiguousarray(x),
            "idxf": idxf,
            "cmbi0": cmbi[0], "cmbi1": cmbi[1],
            "l2i": l2i,
            "recip": recip_pk,
            "xroot": xroot,
            "Wt": np.ascontiguousarray(W.T.astype(np.float32)),
            "bias": b.astype(np.float32).reshape(D, 1),
            "ident": np.eye(P, dtype=np.float32),
        })

    plan = dict(TB=TB, band_nodes=band_nodes, ncap=ncap, PMAX=PMAX, CM=CM,
                band_layout=band_layout, NC_tot=NC_tot, bounds=bounds, counts=counts)
    return plan, in_maps


def _build(plan):
    import concourse.bass as bass
    import concourse.mybir as mybir
    from contextlib import ExitStack

    f32 = mybir.dt.float32
    i32 = mybir.dt.int32
    COPY = mybir.ActivationFunctionType.Copy
    TB = plan["TB"]
    PMAX = plan["PMAX"]
    CM = plan["CM"]
    lay = plan["band_layout"]
    NC0 = lay[0]["NC"]
    NC1 = lay[1]["NC"]
    NC_tot = plan["NC_tot"]
    ncap = plan["ncap"]

    nc = bass.Bass()
    x_d = nc.declare_dram_parameter("x", [N, D], f32, isOutput=False)
    idxf_d = nc.declare_dram_parameter("idxf", [NC_tot, P, SLOTS], i32, isOutput=False)
    cmb_d = [nc.declare_dram_parameter(f"cmbi{b2}", [P, max(1, CM[b2] * PMAX)], i32,
                                       isOutput=False) for b2 in range(2)]
    l2i_d = nc.declare_dram_parameter("l2i", [2, P, TB], i32, isOutput=False)
    recip_d = nc.declare_dram_parameter("recip", [2, P, TB], f32, isOutput=False)
    xroot_d = nc.declare_dram_parameter("xroot", [ncap, D], f32, isOutput=False)
    wt_d = nc.declare_dram_parameter("Wt", [P, D], f32, isOutput=False)
    b_d = nc.declare_dram_parameter("bias", [D, 1], f32, isOutput=False)
    id_d = nc.declare_dram_parameter("ident", [P, P], f32, isOutput=False)
    out_d = nc.declare_dram_parameter("out", [ncap, D], f32, isOutput=True)
    debug = bool(os.environ.get("BASS_KERNEL_DEBUG"))
    if debug:
        dbg_s1 = [nc.declare_dram_parameter(f"dbg_s1_{b2}", [P * lay[b2]["R"], D], f32,
                                            isOutput=True) for b2 in range(2)]
        dbg_agg = [nc.declare_dram_parameter(f"dbg_agg{b2}", [P, TB * D], f32,
                                             isOutput=True) for b2 in range(2)]

    s1_d = [nc.dram_tensor(f"s1_{b2}", [P * lay[b2]["R"], D], f32) for b2 in range(2)]

    ctx = ExitStack()
    sb = lambda nm, shape, dt: ctx.enter_context(nc.sbuf_tensor(nm, shape, dt))
    ps = lambda nm, shape: ctx.enter_context(nc.psum_tensor(nm, shape, f32))

    with ctx:
        g_t = [sb(f"g{i}", [P, SLOTS * D], f32) for i in range(GBUFS)]
        idx_t = [sb(f"ix{i}", [P, SLOTS], i32) for i in range(IBUFS)]
        s1sb = [sb(f"s1sb{i}", [P, lay[i]["R"] * D], f32) for i in range(2)]
        agg = [sb(f"agg{i}", [P, TB * D], f32) for i in range(2)]
        cmbg = [sb(f"cmbg{i}", [P, max(1, CM[i] * PMAX) * D], f32) for i in range(2)]
        cmbi_t = [sb(f"cmbit{i}", [P, max(1, CM[i] * PMAX)], i32) for i in range(2)]
        l2i_t = [sb(f"l2it{i}", [P, TB], i32) for i in range(2)]
        recip_t = [sb(f"recipt{i}", [P, TB], f32) for i in range(2)]
        wt_t = sb("wtt", [P, D], f32)
        bias_t = sb("biast", [D, 1], f32)
        id_t = sb("idt", [P, P], f32)
        scaled = [sb(f"scaled{i}", [P, D], f32) for i in range(2)]
        xr = [sb(f"xr{i}", [P, D], f32) for i in range(2)]
        featT = [sb(f"featT{i}", [P, P], f32) for i in range(2)]
        outb = [sb(f"outb{i}", [D, P], f32) for i in range(2)]
        outs = [sb(f"outs{i}", [P, D], f32) for i in range(2)]
        tp1 = [ps(f"tp1_{i}", [D, P]) for i in range(2)]
        tp2 = [ps(f"tp2_{i}", [D, P]) for i in range(2)]
        mmp = [ps(f"mmp{i}", [D, P]) for i in range(2)]
        outTp = [ps(f"outTp{i}", [P, D]) for i in range(2)]

        with (
            nc.semaphore("hw_sp") as hw_sp,
            nc.semaphore("hw_act") as hw_act,
            nc.semaphore("sw") as sw,
            nc.semaphore("vd") as vd,
            nc.semaphore("ad") as ad,
            nc.semaphore("pe") as pe,
            nc.Block() as block,
        ):
            cnt = dict(hw_sp=0, hw_act=0, sw=0, vd=0, ad=0, pe=0)
            S = dict(sync=[], gpsimd=[], vector=[], scalar=[], tensor=[])
            sem_obj = dict(hw_sp=hw_sp, hw_act=hw_act, sw=sw, vd=vd, ad=ad, pe=pe)

            def emit(eng, fn):
                S[eng].append(fn)

            def wait(eng, sem, val):
                if val > 0:
                    S[eng].append(lambda e, s=sem_obj[sem], v=val: e.wait_ge(s, v))

            def bump(sem, by):
                cnt[sem] += by
                return cnt[sem]

            # ---------- prelude ----------
            def prelude(e):
                e.dma_start(out=wt_t[:], in_=wt_d[:]).then_inc(hw_sp, 16)
                e.dma_start(out=bias_t[:], in_=b_d[:]).then_inc(hw_sp, 16)
                e.dma_start(out=id_t[:], in_=id_d[:]).then_inc(hw_sp, 16)
                e.dma_start(out=recip_t[0][:], in_=recip_d[0]).then_inc(hw_sp, 16)
                e.dma_start(out=recip_t[1][:], in_=recip_d[1]).then_inc(hw_sp, 16)
                e.dma_start(out=l2i_t[0][:], in_=l2i_d[0]).then_inc(hw_sp, 16)
                e.dma_start(out=l2i_t[1][:], in_=l2i_d[1]).then_inc(hw_sp, 16)
            emit("sync", prelude)
            hw_sp_prelude = bump("hw_sp", 7 * 16)

            wait("vector", "hw_sp", hw_sp_prelude)
            def zerocols(e):
                e.memset(s1sb[0][:, 0:D], 0.0)
                e.memset(s1sb[1][:, 0:D], 0.0).then_inc(vd, 1)
            emit("vector", zerocols)
            vd_zero = bump("vd", 1)

            wait("tensor", "hw_sp", hw_sp_prelude)

            # ---------- L1 ----------
            hw_after_idx = {}
            sw_after_chunk = {}
            vd_after_chunk = {}

            def chunk_pieces(beta, ci):
                local = ci - (0 if beta == 0 else NC0)
                return [pc for pc in lay[beta]["pieces"] if pc["chunk"] == local]

            def emit_idx_dma(ci):
                slot = ci % IBUFS
                prev = ci - IBUFS
                if prev >= 0:
                    wait("sync", "sw", sw_after_chunk[prev])
                def f(e, ci=ci, slot=slot):
                    e.dma_start(out=idx_t[slot][:], in_=idxf_d[ci]).then_inc(hw_sp, 16)
                emit("sync", f)
                hw_after_idx[ci] = bump("hw_sp", 16)

            def emit_chunk_gathers(beta, ci):
                slot = ci % IBUFS
                gslot = ci % GBUFS
                wait("gpsimd", "hw_sp", hw_after_idx[ci])
                prevg = ci - GBUFS
                if prevg >= 0:
                    wait("gpsimd", "vd", vd_after_chunk[prevg])
                # HW quirk: the indirect DMA consumes exactly ONE index per
                # partition -> one gather instruction per slot column.
                for pc in chunk_pieces(beta, ci):
                    s0, Wd = pc["col0"], pc["G"] * pc["K"]
                    ph = max(pc["ph"], 2)  # 1-row indirect DMAs unsupported
                    for c in range(s0, s0 + Wd):
                        def f(e, c=c, ph=ph, slot=slot, gslot=gslot):
                            e.indirect_dma_start(
                                out=g_t[gslot][0:ph, c * D:(c + 1) * D],
                                out_offset=None,
                                in_=x_d[:],
                                in_offset=bass.IndirectOffsetOnAxis(
                                    ap=idx_t[slot][0:ph, c:c + 1], axis=0),
                            ).then_inc(sw, 16)
                        emit("gpsimd", f)
                        bump("sw", 16)
                sw_after_chunk[ci] = cnt["sw"]

            def emit_chunk_tree(beta, ci):
                gslot = ci % GBUFS
                wait("vector", "sw", sw_after_chunk[ci])
                pcs = chunk_pieces(beta, ci)
                for pi, pc in enumerate(pcs):
                    K, G, ph, s0, c0 = pc["K"], pc["G"], pc["ph"], pc["col0"], pc["s1c0"]
                    base = g_t[gslot]
                    for (h, w) in _fold_steps(K):
                        def f(e, base=base, ph=ph, G=G, K=K, s0=s0, h=h, w=w):
                            v = base[0:ph, s0 * D:(s0 + G * K) * D].rearrange(
                                "p (g k f) -> p g k f", g=G, f=D)
                            e.tensor_add(out=v[:, :, 0:w - h, :],
                                         in0=v[:, :, 0:w - h, :],
                                         in1=v[:, :, h:w, :])
                        emit("vector", f)
                    is_last = pi == len(pcs) - 1
                    def f(e, base=base, ph=ph, G=G, K=K, s0=s0, c0=c0,
                          beta=beta, is_last=is_last):
                        src = base[0:ph, s0 * D:(s0 + G * K) * D].rearrange(
                            "p (g k f) -> p g k f", g=G, f=D)[:, :, 0, :]
                        dst = s1sb[beta][0:ph, c0 * D:(c0 + G) * D].rearrange(
                            "p (g f) -> p g f", g=G)
                        ins = e.tensor_copy(out=dst, in_=src)
                        if is_last:
                            ins.then_inc(vd, 1)
                    emit("vector", f)
                vd_after_chunk[ci] = bump("vd", 1)

            # ---------- band epilogue ----------
            sw_after_l2 = {}

            def emit_epilogue(beta):
                R = lay[beta]["R"]
                last_ci = (NC0 - 1) if beta == 0 else (NC0 + NC1 - 1)
                cc0 = lay[beta]["cmb_col0"]
                wait("sync", "vd", max(vd_after_chunk[last_ci], vd_zero))
                def f(e, beta=beta, R=R, cc0=cc0):
                    dst = s1_d[beta][:].rearrange("(p r) f -> p r f", r=R)[:, 0:cc0, :]
                    src = s1sb[beta][:, 0:cc0 * D].rearrange("p (r f) -> p r f", f=D)
                    e.dma_start(out=dst, in_=src).then_inc(hw_sp, 16)
                emit("sync", f)
                hw_s1w = bump("hw_sp", 16)
                if CM[beta]:
                    def f(e, beta=beta):
                        e.dma_start(out=cmbi_t[beta][:], in_=cmb_d[beta][:]).then_inc(hw_sp, 16)
                    emit("sync", f)
                    hw_cmbi = bump("hw_sp", 16)
                    wait("gpsimd", "hw_sp", hw_cmbi)
                    for c in range(CM[beta] * PMAX):
                        def f(e, beta=beta, c=c):
                            e.indirect_dma_start(
                                out=cmbg[beta][:, c * D:(c + 1) * D],
                                out_offset=None,
                                in_=s1_d[beta][:],
                                in_offset=bass.IndirectOffsetOnAxis(
                                    ap=cmbi_t[beta][:, c:c + 1], axis=0),
                            ).then_inc(sw, 16)
                        emit("gpsimd", f)
                        bump("sw", 16)
                    sw_cmbg = cnt["sw"]
                    wait("vector", "sw", sw_cmbg)
                    for (h, w) in _fold_steps(PMAX):
                        def f(e, beta=beta, h=h, w=w):
                            v = cmbg[beta][:, 0:CM[beta] * PMAX * D].rearrange(
                                "p (m k f) -> p m k f", m=CM[beta], f=D)
                            e.tensor_add(out=v[:, :, 0:w - h, :],
                                         in0=v[:, :, 0:w - h, :],
                                         in1=v[:, :, h:w, :])
                        emit("vector", f)
                    def f(e, beta=beta, cc0=cc0):
                        src = cmbg[beta][:, 0:CM[beta] * PMAX * D].rearrange(
                            "p (m k f) -> p m k f", m=CM[beta], f=D)[:, :, 0, :]
                        dst = s1sb[beta][:, cc0 * D:(cc0 + CM[beta]) * D].rearrange(
                            "p (m f) -> p m f", f=D)
                        e.tensor_copy(out=dst, in_=src).then_inc(vd, 1)
                    emit("vector", f)
                    vd_cmbf = bump("vd", 1)
                    wait("sync", "vd", vd_cmbf)
                    def f(e, beta=beta, R=R, cc0=cc0):
                        dst = s1_d[beta][:].rearrange("(p r) f -> p r f", r=R)[:, cc0:cc0 + CM[beta], :]
                        src = s1sb[beta][:, cc0 * D:(cc0 + CM[beta]) * D].rearrange(
                            "p (m f) -> p m f", f=D)
                        e.dma_start(out=dst, in_=src).then_inc(hw_sp, 16)
                    emit("sync", f)
                    hw_s1w2 = bump("hw_sp", 16)
                    wait("gpsimd", "hw_sp", hw_s1w2)
                else:
                    wait("gpsimd", "hw_sp", hw_s1w)
                for j in range(TB):
                    def f(e, beta=beta, j=j):
                        e.indirect_dma_start(
                            out=agg[beta][:, j * D:(j + 1) * D],
                            out_offset=None,
                            in_=s1_d[beta][:],
                            in_offset=bass.IndirectOffsetOnAxis(
                                ap=l2i_t[beta][:, j:j + 1], axis=0),
                        ).then_inc(sw, 16)
                    emit("gpsimd", f)
                    bump("sw", 16)
                sw_after_l2[beta] = cnt["sw"]

            # ---------- final phase ----------
            pe_t1, pe_t2, pe_mm, pe_t3 = {}, {}, {}, {}
            ad_b2, ad_c = {}, {}
            hw_xr, hw_out = {}, {}
            vd_scale, vd_bias = {}, {}

            def emit_final_tile(beta, j):
                t = beta * TB + j
                s = t % 2
                # DVE: scale by 1/deg
                if j == 0:
                    wait("vector", "sw", sw_after_l2[beta])
                if t - 2 >= 0:
                    wait("vector", "pe", pe_t1[t - 2])   # WAR scaled slot
                def f(e, beta=beta, j=j, s=s):
                    e.tensor_scalar_mul(scaled[s][:], agg[beta][:, j * D:(j + 1) * D],
                                        recip_t[beta][:, j:j + 1]).then_inc(vd, 1)
                emit("vector", f)
                vd_scale[t] = bump("vd", 1)
                # ACT: xroot load
                if t - 2 >= 0:
                    wait("scalar", "pe", pe_t2[t - 2])   # WAR xr slot
                def f(e, t=t, s=s):
                    e.dma_start(out=xr[s][:], in_=xroot_d[t * P:(t + 1) * P, :]).then_inc(hw_act, 16)
                emit("scalar", f)
                hw_xr[t] = bump("hw_act", 16)
                # PE: T1
                wait("tensor", "vd", vd_scale[t])
                if t - 2 >= 0:
                    wait("tensor", "ad", ad_b2[t - 2])   # WAR tp1/tp2 psum + featT
                def f(e, s=s):
                    e.matmul(tp1[s][:], lhsT=scaled[s][:], rhs=id_t[:],
                             is_transpose=True, start=True, stop=True).then_inc(pe, 1)
                emit("tensor", f)
                pe_t1[t] = bump("pe", 1)
                # ACT: copyA
                wait("scalar", "pe", pe_t1[t])
                def f(e, s=s):
                    e.activation(featT[s][0:D, :], tp1[s][:], COPY).then_inc(ad, 1)
                emit("scalar", f)
                bump("ad", 1)
                # PE: T2
                wait("tensor", "hw_act", hw_xr[t])
                def f(e, s=s):
                    e.matmul(tp2[s][:], lhsT=xr[s][:], rhs=id_t[:],
                             is_transpose=True, start=True, stop=True).then_inc(pe, 1)
                emit("tensor", f)
                pe_t2[t] = bump("pe", 1)
                # ACT: copyB
                wait("scalar", "pe", pe_t2[t])
                def f(e, s=s):
                    e.activation(featT[s][D:P, :], tp2[s][:], COPY).then_inc(ad, 1)
                emit("scalar", f)
                ad_b2[t] = bump("ad", 1)
                # PE: MM (feat -> out, accumul. none)
                wait("tensor", "ad", ad_b2[t])
                if t - 2 >= 0:
                    wait("tensor", "vd", vd_bias[t - 2])  # WAR mmp psum
                def f(e, s=s):
                    e.matmul(mmp[s][:], lhsT=wt_t[:], rhs=featT[s][:],
                             start=True, stop=True).then_inc(pe, 1)
                emit("tensor", f)
                pe_mm[t] = bump("pe", 1)
                # DVE: bias add
                wait("vector", "pe", pe_mm[t])
                def f(e, s=s):
                    e.tensor_add(out=outb[s][:], in0=mmp[s][:],
                                 in1=bias_t[:].to_broadcast([D, P])).then_inc(vd, 1)
                emit("vector", f)
                vd_bias[t] = bump("vd", 1)
                # PE: T3
                wait("tensor", "vd", vd_bias[t])
                if t - 2 >= 0:
                    wait("tensor", "ad", ad_c[t - 2])     # WAR outTp psum
                def f(e, s=s):
                    e.matmul(outTp[s][:], lhsT=outb[s][:], rhs=id_t[0:D, 0:D],
                             is_transpose=True, start=True, stop=True).then_inc(pe, 1)
                emit("tensor", f)
                pe_t3[t] = bump("pe", 1)
                # ACT: copyC + out store
                wait("scalar", "pe", pe_t3[t])
                if t - 2 >= 0:
                    wait("scalar", "hw_act", hw_out[t - 2])  # WAR outs slot
                def f(e, s=s):
                    e.activation(outs[s][:], outTp[s][:], COPY).then_inc(ad, 1)
                emit("scalar", f)
                ad_c[t] = bump("ad", 1)
                def f(e, t=t, s=s):
                    e.dma_start(out=out_d[t * P:(t + 1) * P, :], in_=outs[s][:]).then_inc(hw_act, 16)
                emit("scalar", f)
                hw_out[t] = bump("hw_act", 16)

            # ---------- global order ----------
            for ci in range(NC0):
                emit_idx_dma(ci)
                emit_chunk_gathers(0, ci)
                emit_chunk_tree(0, ci)
            pre1 = min(2, NC1)
            for ci in range(NC0, NC0 + pre1):
                emit_idx_dma(ci)
                emit_chunk_gathers(1, ci)
                emit_chunk_tree(1, ci)
            emit_epilogue(0)
            for j in range(TB):
                emit_final_tile(0, j)
            for ci in range(NC0 + pre1, NC0 + NC1):
                emit_idx_dma(ci)
                emit_chunk_gathers(1, ci)
                emit_chunk_tree(1, ci)
            emit_epilogue(1)
            for j in range(TB):
                emit_final_tile(1, j)

            if debug:
                # dump S1 (via DRAM->DRAM copy) and agg tiles
                wait("sync", "sw", cnt["sw"])
                wait("sync", "vd", cnt["vd"])
                def fdbg(e):
                    e.dma_start(out=dbg_s1[0][:], in_=s1_d[0][:]).then_inc(hw_sp, 16)
                    e.dma_start(out=dbg_s1[1][:], in_=s1_d[1][:]).then_inc(hw_sp, 16)
                    e.dma_start(out=dbg_agg[0][:], in_=agg[0][:]).then_inc(hw_sp, 16)
                    e.dma_start(out=dbg_agg[1][:], in_=agg[1][:]).then_inc(hw_sp, 16)
                emit("sync", fdbg)
                bump("hw_sp", 64)

            @block.sync
            def _(eng):
                for f in S["sync"]:
                    f(eng)

            @block.gpsimd
            def _(eng):
                for f in S["gpsimd"]:
                    f(eng)

            @block.vector
            def _(eng):
                for f in S["vector"]:
                    f(eng)

            @block.scalar
            def _(eng):
                for f in S["scalar"]:
                    f(eng)

            @block.tensor
            def _(eng):
                for f in S["tensor"]:
                    f(eng)

    return nc


def kernel(x, row, colptr, W, b):
    global LAST_EXEC_TIME_NS, LAST_MEAN_EXEC_TIME_NS
    from concourse.bass_utils import run_bass_kernel_spmd

    x = np.asarray(x, np.float32)
    row = np.asarray(row, np.int32)
    colptr = np.asarray(colptr, np.int32)
    W = np.asarray(W, np.float32)
    b = np.asarray(b, np.float32)

    plan, in_maps = _plan_and_arrays(x, row, colptr, W, b)
    nc = _build(plan)

    # ensure the axon (neuron) backend is active even if the caller switched
    # jax to cpu (e.g. to run the reference)
    import jax
    if not any(d.platform != "cpu" for d in jax.devices()):
        jax.config.update("jax_platforms", "axon,cpu")
        from jax._src import xla_bridge
        xla_bridge._clear_backends()
        try:
            jax.clear_caches()
        except Exception:
            pass

    trace = bool(os.environ.get("BASS_KERNEL_TRACE"))
    kwargs = {}
    if trace:
        # the NTFF profile hook requires a live axon client connection
        import jax.numpy as jnp
        dev = [d for d in jax.devices() if d.platform != "cpu"][0]
        jax.jit(lambda a: a + 1)(jax.device_put(jnp.zeros((8,)), dev)).block_until_ready()
        kwargs = dict(trace=True, trace_cores=list(range(NCORES)))
    res = run_bass_kernel_spmd(nc, in_maps, list(range(NCORES)), **kwargs)
    LAST_EXEC_TIME_NS = res.exec_time_ns
    LAST_MEAN_EXEC_TIME_NS = res.mean_exec_time_ns
    globals()["LAST_RESULTS"] = res.results
    globals()["LAST_PLAN"] = plan
    globals()["LAST_IN_MAPS"] = in_maps

    out = np.empty((N, D), np.float32)
    bounds = plan["bounds"]
    for c in range(NCORES):
        n0, n1 = int(bounds[c]), int(bounds[c + 1])
        out[n0:n1] = res.results[c]["out"][:n1 - n0]
    return out



# revision 7
# speedup vs baseline: 1.0013x; 1.0013x over previous
"""CuGraphSAGEConv (mean aggregation) on 8 trn2 NeuronCores via raw Bass.

Strategy
--------
Target nodes are sharded across 8 cores (edge-balanced CSC split); x is
replicated. Per core, each node's degree is decomposed exactly into parts
from a fixed class set (no wasted gather descriptors). Per (band, class):
one or two indirect-DMA gathers pull x[row[e]] rows into SBUF laid out as
[partition, group, K, 64]; a log-depth DVE fold reduces each K-group to a
partial node sum; partials are compacted and staged to DRAM (S1).
Multi-part nodes get a small second gather+fold ("combine") whose result
is written back to S1. A final gather collects one S1 row per node in
natural order; scale by 1/deg, PE-transpose, one matmul with W^T
(128->64), bias, transpose back, contiguous store.

The walrus build in this environment only accepts ONE attached sync-wait
per instruction, so everything is raw Bass with standalone wait_ge's.
Indirect-DMA facts established by probing the hardware:
  - idx tile [P, M] gathers row idx[p, m] into out[p, m*64:(m+1)*64]
    (p-major flat order); dest may be a partition/column slice.
  - out-of-bounds skipping only works for a tail suffix, so all padding
    indices here are in-bounds (row 0) and their results are discarded.
  - compute_op=add accumulates exactly.
"""

import os
import numpy as np

N = 100_000
E = 1_600_000
D = 64
NCORES = 8
P = 128

# exact degree decomposition classes (descending)
CLASSES = list(range(64, 32, -4)) + list(range(32, 0, -1))
SLOTS = 128          # gather-slot capacity per partition per chunk
GBUFS = 3            # chunk gather tile buffers
IBUFS = 3            # idx tile buffers
SPLIT_PIECE_MIN_K = 13  # classes >= this get a separate remainder gather

LAST_EXEC_TIME_NS = None
LAST_MEAN_EXEC_TIME_NS = None


def _decompose(d):
    parts = []
    while d > 64:
        parts.append(64)
        d -= 64
    if d > 32:
        b = d - (d % 4) if d >= 36 else 32
        parts.append(b)
        d -= b
    if d > 0:
        parts.append(d)
    return parts


def _fold_steps(w):
    steps = []
    while w > 1:
        h = 1 << ((w - 1).bit_length() - 1)
        if h == w:
            h = w // 2
        steps.append((h, w))
        w = h
    return steps


def _plan_and_arrays(x, row, colptr, W, b):
    deg = np.diff(colptr).astype(np.int64)

    cuts = np.searchsorted(colptr, (np.arange(1, NCORES) * (E // NCORES)).astype(colptr.dtype))
    bounds = np.concatenate([[0], cuts, [N]]).astype(np.int64)
    counts = np.diff(bounds)
    maxcount = int(counts.max())

    TB = (maxcount + 255) // 256
    band_nodes = TB * P
    ncap = 2 * band_nodes

    core_members = []
    core_nparts = []
    for c in range(NCORES):
        n0, n1 = int(bounds[c]), int(bounds[c + 1])
        cnt = n1 - n0
        dg = deg[n0:n1]
        npart = np.zeros(max(cnt, 1), np.int64)
        bands = [dict(), dict()]
        for loc in range(cnt):
            d = int(dg[loc])
            if d == 0:
                continue
            parts = _decompose(d)
            npart[loc] = len(parts)
            beta = 0 if loc < band_nodes else 1
            es = int(colptr[n0 + loc])
            for k, K in enumerate(parts):
                bands[beta].setdefault(K, []).append((loc, es, k))
                es += K
        core_members.append(bands)
        core_nparts.append(npart)

    PMAX = max(2, max(int(cn.max()) for cn in core_nparts))

    nmax = [dict(), dict()]
    for beta in range(2):
        for K in CLASSES:
            m = max(len(core_members[c][beta].get(K, [])) for c in range(NCORES))
            if m:
                nmax[beta][K] = m

    ncmb_max = [0, 0]
    for beta in range(2):
        for c in range(NCORES):
            npc = core_nparts[c]
            lo = beta * band_nodes
            hi = min(int(counts[c]), (beta + 1) * band_nodes)
            if hi > lo:
                ncmb_max[beta] = max(ncmb_max[beta], int((npc[lo:hi] >= 2).sum()))
    CM = [((ncmb_max[b] + P - 1) // P) if ncmb_max[b] else 0 for b in range(2)]

    band_layout = []
    for beta in range(2):
        pieces = []
        for K in CLASSES:
            n = nmax[beta].get(K, 0)
            if n == 0:
                continue
            if K >= SPLIT_PIECE_MIN_K:
                G = n // P
                r = n % P
                off = 0
                while G > 0:
                    g = min(G, SLOTS // K)
                    pieces.append(dict(K=K, ph=P, G=g, kind="bulk", mem_off=off))
                    off += P * g
                    G -= g
                if r:
                    pieces.append(dict(K=K, ph=r, G=1, kind="rem", mem_off=off))
            else:
                G1 = (n + P - 1) // P
                off = 0
                while G1 > 0:
                    g = min(G1, SLOTS // K)
                    pieces.append(dict(K=K, ph=P, G=g, kind="pad", mem_off=off))
                    off += P * g
                    G1 -= g
        chunk_cols = []
        for pc in pieces:
            w = pc["G"] * pc["K"]
            for ci, used in enumerate(chunk_cols):
                if used + w <= SLOTS:
                    pc["chunk"] = ci
                    pc["col0"] = used
                    chunk_cols[ci] = used + w
                    break
            else:
                pc["chunk"] = len(chunk_cols)
                pc["col0"] = 0
                chunk_cols.append(w)
        NC = len(chunk_cols)
        s1c = 1
        for pc in pieces:
            pc["s1c0"] = s1c
            s1c += pc["G"]
        cmb_col0 = s1c
        R = s1c + CM[beta]
        band_layout.append(dict(pieces=pieces, NC=NC, R=R, cmb_col0=cmb_col0))

    NC_tot = band_layout[0]["NC"] + band_layout[1]["NC"]

    row32 = row.astype(np.int32, copy=False)
    in_maps = []
    for c in range(NCORES):
        n0 = int(bounds[c])
        cnt = int(counts[c])
        idxf = np.zeros((NC_tot, P, SLOTS), np.int32)
        parts_rows = np.zeros((ncap, PMAX), np.int32)
        parts_cnt = np.zeros(ncap, np.int32)

        for beta in range(2):
            lay = band_layout[beta]
            R = lay["R"]
            chunk_base = 0 if beta == 0 else band_layout[0]["NC"]
            for K in CLASSES:
                if K not in nmax[beta]:
                    continue
                mem = core_members[c][beta].get(K, [])
                n = len(mem)
                if n:
                    nodes = np.array([m[0] for m in mem], np.int64)
                    es = np.array([m[1] for m in mem], np.int64)
                    ords = np.array([m[2] for m in mem], np.int64)
                    vals = row32[(es[:, None] + np.arange(K)).reshape(-1)].reshape(n, K)
                for pc in lay["pieces"]:
                    if pc["K"] != K:
                        continue
                    o = pc["mem_off"]
                    ncells = pc["ph"] * pc["G"]
                    a = np.zeros((ncells, K), np.int32)
                    take = max(0, min(n - o, ncells)) if n else 0
                    if take:
                        a[:take] = vals[o:o + take]
                    ci = chunk_base + pc["chunk"]
                    s0 = pc["col0"]
                    if pc["kind"] == "rem":
                        idxf[ci, :pc["ph"], s0:s0 + K] = a
                    else:
                        idxf[ci, :, s0:s0 + pc["G"] * K] = a.reshape(P, pc["G"] * K)
                    if take:
                        j = np.arange(take)
                        if pc["kind"] == "rem":
                            rows = j * R + pc["s1c0"]
                        else:
                            rows = (j // pc["G"]) * R + pc["s1c0"] + (j % pc["G"])
                        nd = nodes[o:o + take]
                        od = ords[o:o + take]
                        parts_rows[nd, od] = rows.astype(np.int32)
                        parts_cnt[nd] = np.maximum(parts_cnt[nd], (od + 1).astype(np.int32))

        cmbi = []
        l2i = np.zeros((2, P, TB), np.int32)
        for beta in range(2):
            lay = band_layout[beta]
            R = lay["R"]
            cmw = max(1, CM[beta] * PMAX)
            ci_arr = np.zeros((P, cmw), np.int32)
            lo = beta * band_nodes
            hi = min(cnt, (beta + 1) * band_nodes)
            node_row = np.zeros(band_nodes, np.int32)
            if hi > lo:
                loc = np.arange(lo, hi)
                pcn = parts_cnt[lo:hi]
                single = pcn == 1
                node_row[:hi - lo][single] = parts_rows[lo:hi][single, 0]
                multi_nodes = loc[pcn >= 2]
                for m, nd in enumerate(multi_nodes):
                    p_ = m // CM[beta]
                    jm = m % CM[beta]
                    for k in range(int(parts_cnt[nd])):
                        ci_arr[p_, jm * PMAX + k] = parts_rows[nd, k]
                    node_row[nd - lo] = p_ * R + lay["cmb_col0"] + jm
            cmbi.append(ci_arr)
            l2i[beta] = node_row.reshape(TB, P).T

        recip = np.ones(ncap, np.float32)
        recip[:cnt] = 1.0 / np.maximum(deg[n0:n0 + cnt].astype(np.float32), 1.0)
        recip_pk = recip.reshape(2, TB, P).transpose(0, 2, 1).copy()
        xroot = np.zeros((ncap, D), np.float32)
        xroot[:cnt] = x[n0:n0 + cnt]

        in_maps.append({
            "x": np.ascontiguousarray(x),
            "idxf": idxf,
            "cmbi0": cmbi[0], "cmbi1": cmbi[1],
            "l2i": l2i,
            "recip": recip_pk,
            "xrootT": np.ascontiguousarray(xroot.T),
            "Wt": np.ascontiguousarray(W.T.astype(np.float32)),
            "bias": b.astype(np.float32).reshape(D, 1),
            "ident": np.eye(P, dtype=np.float32),
        })

    plan = dict(TB=TB, band_nodes=band_nodes, ncap=ncap, PMAX=PMAX, CM=CM,
                band_layout=band_layout, NC_tot=NC_tot, bounds=bounds, counts=counts)
    return plan, in_maps


def _build(plan):
    import concourse.bass as bass
    import concourse.mybir as mybir
    from contextlib import ExitStack

    f32 = mybir.dt.float32
    i32 = mybir.dt.int32
    COPY = mybir.ActivationFunctionType.Copy
    TB = plan["TB"]
    PMAX = plan["PMAX"]
    CM = plan["CM"]
    lay = plan["band_layout"]
    NC0 = lay[0]["NC"]
    NC1 = lay[1]["NC"]
    NC_tot = plan["NC_tot"]
    ncap = plan["ncap"]

    nc = bass.Bass()
    x_d = nc.declare_dram_parameter("x", [N, D], f32, isOutput=False)
    idxf_d = nc.declare_dram_parameter("idxf", [NC_tot, P, SLOTS], i32, isOutput=False)
    cmb_d = [nc.declare_dram_parameter(f"cmbi{b2}", [P, max(1, CM[b2] * PMAX)], i32,
                                       isOutput=False) for b2 in range(2)]
    l2i_d = nc.declare_dram_parameter("l2i", [2, P, TB], i32, isOutput=False)
    recip_d = nc.declare_dram_parameter("recip", [2, P, TB], f32, isOutput=False)
    xrootT_d = nc.declare_dram_parameter("xrootT", [D, ncap], f32, isOutput=False)
    wt_d = nc.declare_dram_parameter("Wt", [P, D], f32, isOutput=False)
    b_d = nc.declare_dram_parameter("bias", [D, 1], f32, isOutput=False)
    id_d = nc.declare_dram_parameter("ident", [P, P], f32, isOutput=False)
    outT_d = nc.declare_dram_parameter("outT", [D, ncap], f32, isOutput=True)
    debug = bool(os.environ.get("BASS_KERNEL_DEBUG"))
    if debug:
        dbg_s1 = [nc.declare_dram_parameter(f"dbg_s1_{b2}", [P * lay[b2]["R"], D], f32,
                                            isOutput=True) for b2 in range(2)]
        dbg_agg = [nc.declare_dram_parameter(f"dbg_agg{b2}", [P, TB * D], f32,
                                             isOutput=True) for b2 in range(2)]

    s1_d = [nc.dram_tensor(f"s1_{b2}", [P * lay[b2]["R"], D], f32) for b2 in range(2)]

    ctx = ExitStack()
    sb = lambda nm, shape, dt: ctx.enter_context(nc.sbuf_tensor(nm, shape, dt))
    ps = lambda nm, shape: ctx.enter_context(nc.psum_tensor(nm, shape, f32))

    with ctx:
        g_t = [sb(f"g{i}", [P, SLOTS * D], f32) for i in range(GBUFS)]
        idx_t = [sb(f"ix{i}", [P, SLOTS], i32) for i in range(IBUFS)]
        s1sb = [sb(f"s1sb{i}", [P, lay[i]["R"] * D], f32) for i in range(2)]
        agg = [sb(f"agg{i}", [P, TB * D], f32) for i in range(2)]
        cmbg = [sb(f"cmbg{i}", [P, max(1, CM[i] * PMAX) * D], f32) for i in range(2)]
        cmbi_t = [sb(f"cmbit{i}", [P, max(1, CM[i] * PMAX)], i32) for i in range(2)]
        l2i_t = [sb(f"l2it{i}", [P, TB], i32) for i in range(2)]
        recip_t = [sb(f"recipt{i}", [P, TB], f32) for i in range(2)]
        wt_t = sb("wtt", [P, D], f32)
        bias_t = sb("biast", [D, 1], f32)
        id_t = sb("idt", [P, P], f32)
        scaled = [sb(f"scaled{i}", [P, D], f32) for i in range(2)]
        featT = [sb(f"featT{i}", [P, P], f32) for i in range(2)]
        outb = [sb(f"outb{i}", [D, P], f32) for i in range(2)]
        tp1 = [ps(f"tp1_{i}", [D, P]) for i in range(2)]
        mmp = [ps(f"mmp{i}", [D, P]) for i in range(2)]

        with (
            nc.semaphore("hw_sp") as hw_sp,
            nc.semaphore("hw_act") as hw_act,
            nc.semaphore("sw") as sw,
            nc.semaphore("vd") as vd,
            nc.semaphore("ad") as ad,
            nc.semaphore("pe") as pe,
            nc.Block() as block,
        ):
            cnt = dict(hw_sp=0, hw_act=0, sw=0, vd=0, ad=0, pe=0)
            S = dict(sync=[], gpsimd=[], vector=[], scalar=[], tensor=[])
            sem_obj = dict(hw_sp=hw_sp, hw_act=hw_act, sw=sw, vd=vd, ad=ad, pe=pe)

            def emit(eng, fn):
                S[eng].append(fn)

            def wait(eng, sem, val):
                if val > 0:
                    S[eng].append(lambda e, s=sem_obj[sem], v=val: e.wait_ge(s, v))

            def bump(sem, by):
                cnt[sem] += by
                return cnt[sem]

            # ---------- prelude ----------
            def prelude(e):
                e.dma_start(out=wt_t[:], in_=wt_d[:]).then_inc(hw_sp, 16)
                e.dma_start(out=bias_t[:], in_=b_d[:]).then_inc(hw_sp, 16)
                e.dma_start(out=id_t[:], in_=id_d[:]).then_inc(hw_sp, 16)
                e.dma_start(out=recip_t[0][:], in_=recip_d[0]).then_inc(hw_sp, 16)
                e.dma_start(out=recip_t[1][:], in_=recip_d[1]).then_inc(hw_sp, 16)
                e.dma_start(out=l2i_t[0][:], in_=l2i_d[0]).then_inc(hw_sp, 16)
                e.dma_start(out=l2i_t[1][:], in_=l2i_d[1]).then_inc(hw_sp, 16)
            emit("sync", prelude)
            hw_sp_prelude = bump("hw_sp", 7 * 16)

            wait("vector", "hw_sp", hw_sp_prelude)
            def zerocols(e):
                e.memset(s1sb[0][:, 0:D], 0.0)
                e.memset(s1sb[1][:, 0:D], 0.0).then_inc(vd, 1)
            emit("vector", zerocols)
            vd_zero = bump("vd", 1)

            wait("tensor", "hw_sp", hw_sp_prelude)

            # ---------- L1 ----------
            hw_after_idx = {}
            sw_after_chunk = {}
            vd_after_chunk = {}

            def chunk_pieces(beta, ci):
                local = ci - (0 if beta == 0 else NC0)
                return [pc for pc in lay[beta]["pieces"] if pc["chunk"] == local]

            def emit_idx_dma(ci):
                slot = ci % IBUFS
                prev = ci - IBUFS
                if prev >= 0:
                    wait("sync", "sw", sw_after_chunk[prev])
                def f(e, ci=ci, slot=slot):
                    e.dma_start(out=idx_t[slot][:], in_=idxf_d[ci]).then_inc(hw_sp, 16)
                emit("sync", f)
                hw_after_idx[ci] = bump("hw_sp", 16)

            def emit_chunk_gathers(beta, ci):
                slot = ci % IBUFS
                gslot = ci % GBUFS
                wait("gpsimd", "hw_sp", hw_after_idx[ci])
                prevg = ci - GBUFS
                if prevg >= 0:
                    wait("gpsimd", "vd", vd_after_chunk[prevg])
                # HW quirk: the indirect DMA consumes exactly ONE index per
                # partition -> one gather instruction per slot column.
                for pc in chunk_pieces(beta, ci):
                    s0, Wd = pc["col0"], pc["G"] * pc["K"]
                    ph = max(pc["ph"], 2)  # 1-row indirect DMAs unsupported
                    for c in range(s0, s0 + Wd):
                        def f(e, c=c, ph=ph, slot=slot, gslot=gslot):
                            e.indirect_dma_start(
                                out=g_t[gslot][0:ph, c * D:(c + 1) * D],
                                out_offset=None,
                                in_=x_d[:],
                                in_offset=bass.IndirectOffsetOnAxis(
                                    ap=idx_t[slot][0:ph, c:c + 1], axis=0),
                            ).then_inc(sw, 16)
                        emit("gpsimd", f)
                        bump("sw", 16)
                sw_after_chunk[ci] = cnt["sw"]

            def emit_chunk_tree(beta, ci):
                gslot = ci % GBUFS
                wait("vector", "sw", sw_after_chunk[ci])
                pcs = chunk_pieces(beta, ci)
                for pi, pc in enumerate(pcs):
                    K, G, ph, s0, c0 = pc["K"], pc["G"], pc["ph"], pc["col0"], pc["s1c0"]
                    base = g_t[gslot]
                    for (h, w) in _fold_steps(K):
                        def f(e, base=base, ph=ph, G=G, K=K, s0=s0, h=h, w=w):
                            v = base[0:ph, s0 * D:(s0 + G * K) * D].rearrange(
                                "p (g k f) -> p g k f", g=G, f=D)
                            e.tensor_add(out=v[:, :, 0:w - h, :],
                                         in0=v[:, :, 0:w - h, :],
                                         in1=v[:, :, h:w, :])
                        emit("vector", f)
                    is_last = pi == len(pcs) - 1
                    def f(e, base=base, ph=ph, G=G, K=K, s0=s0, c0=c0,
                          beta=beta, is_last=is_last):
                        src = base[0:ph, s0 * D:(s0 + G * K) * D].rearrange(
                            "p (g k f) -> p g k f", g=G, f=D)[:, :, 0, :]
                        dst = s1sb[beta][0:ph, c0 * D:(c0 + G) * D].rearrange(
                            "p (g f) -> p g f", g=G)
                        ins = e.tensor_copy(out=dst, in_=src)
                        if is_last:
                            ins.then_inc(vd, 1)
                    emit("vector", f)
                vd_after_chunk[ci] = bump("vd", 1)

            # ---------- band epilogue ----------
            sw_after_l2 = {}

            def emit_epilogue(beta):
                R = lay[beta]["R"]
                last_ci = (NC0 - 1) if beta == 0 else (NC0 + NC1 - 1)
                cc0 = lay[beta]["cmb_col0"]
                wait("sync", "vd", max(vd_after_chunk[last_ci], vd_zero))
                def f(e, beta=beta, R=R, cc0=cc0):
                    dst = s1_d[beta][:].rearrange("(p r) f -> p r f", r=R)[:, 0:cc0, :]
                    src = s1sb[beta][:, 0:cc0 * D].rearrange("p (r f) -> p r f", f=D)
                    e.dma_start(out=dst, in_=src).then_inc(hw_sp, 16)
                emit("sync", f)
                hw_s1w = bump("hw_sp", 16)
                if CM[beta]:
                    def f(e, beta=beta):
                        e.dma_start(out=cmbi_t[beta][:], in_=cmb_d[beta][:]).then_inc(hw_sp, 16)
                    emit("sync", f)
                    hw_cmbi = bump("hw_sp", 16)
                    wait("gpsimd", "hw_sp", hw_cmbi)
                    for c in range(CM[beta] * PMAX):
                        def f(e, beta=beta, c=c):
                            e.indirect_dma_start(
                                out=cmbg[beta][:, c * D:(c + 1) * D],
                                out_offset=None,
                                in_=s1_d[beta][:],
                                in_offset=bass.IndirectOffsetOnAxis(
                                    ap=cmbi_t[beta][:, c:c + 1], axis=0),
                            ).then_inc(sw, 16)
                        emit("gpsimd", f)
                        bump("sw", 16)
                    sw_cmbg = cnt["sw"]
                    wait("vector", "sw", sw_cmbg)
                    for (h, w) in _fold_steps(PMAX):
                        def f(e, beta=beta, h=h, w=w):
                            v = cmbg[beta][:, 0:CM[beta] * PMAX * D].rearrange(
                                "p (m k f) -> p m k f", m=CM[beta], f=D)
                            e.tensor_add(out=v[:, :, 0:w - h, :],
                                         in0=v[:, :, 0:w - h, :],
                                         in1=v[:, :, h:w, :])
                        emit("vector", f)
                    def f(e, beta=beta, cc0=cc0):
                        src = cmbg[beta][:, 0:CM[beta] * PMAX * D].rearrange(
                            "p (m k f) -> p m k f", m=CM[beta], f=D)[:, :, 0, :]
                        dst = s1sb[beta][:, cc0 * D:(cc0 + CM[beta]) * D].rearrange(
                            "p (m f) -> p m f", f=D)
                        e.tensor_copy(out=dst, in_=src).then_inc(vd, 1)
                    emit("vector", f)
                    vd_cmbf = bump("vd", 1)
                    wait("sync", "vd", vd_cmbf)
                    def f(e, beta=beta, R=R, cc0=cc0):
                        dst = s1_d[beta][:].rearrange("(p r) f -> p r f", r=R)[:, cc0:cc0 + CM[beta], :]
                        src = s1sb[beta][:, cc0 * D:(cc0 + CM[beta]) * D].rearrange(
                            "p (m f) -> p m f", f=D)
                        e.dma_start(out=dst, in_=src).then_inc(hw_sp, 16)
                    emit("sync", f)
                    hw_s1w2 = bump("hw_sp", 16)
                    wait("gpsimd", "hw_sp", hw_s1w2)
                else:
                    wait("gpsimd", "hw_sp", hw_s1w)
                for j in range(TB):
                    def f(e, beta=beta, j=j):
                        e.indirect_dma_start(
                            out=agg[beta][:, j * D:(j + 1) * D],
                            out_offset=None,
                            in_=s1_d[beta][:],
                            in_offset=bass.IndirectOffsetOnAxis(
                                ap=l2i_t[beta][:, j:j + 1], axis=0),
                        ).then_inc(sw, 16)
                    emit("gpsimd", f)
                    bump("sw", 16)
                sw_after_l2[beta] = cnt["sw"]

            # ---------- final phase ----------
            # Per tile: DVE scale -> PE transpose(agg) -> ACT copy to featT
            # top half; xrootT tile DMA'd (host-pretransposed) into featT
            # bottom half; PE matmul W^T @ featT; DVE bias; store the [D, P]
            # result TRANSPOSED to outT (host un-transposes after readback).
            pe_t1, pe_mm = {}, {}
            ad_cpA = {}
            hw_xrt, hw_out = {}, {}
            vd_scale, vd_bias = {}, {}

            def emit_final_tile(beta, j):
                t = beta * TB + j
                s = t % 2
                # DVE: scale by 1/deg
                if j == 0:
                    wait("vector", "sw", sw_after_l2[beta])
                if t - 2 >= 0:
                    wait("vector", "pe", pe_t1[t - 2])   # WAR scaled slot
                def f(e, beta=beta, j=j, s=s):
                    e.tensor_scalar_mul(scaled[s][:], agg[beta][:, j * D:(j + 1) * D],
                                        recip_t[beta][:, j:j + 1]).then_inc(vd, 1)
                emit("vector", f)
                vd_scale[t] = bump("vd", 1)
                # SYNC: xrootT tile load into featT bottom half
                if t - 2 >= 0:
                    wait("sync", "pe", pe_mm[t - 2])     # WAR featT slot
                def f(e, t=t, s=s):
                    e.dma_start(out=featT[s][D:P, :],
                                in_=xrootT_d[:, t * P:(t + 1) * P]).then_inc(hw_sp, 16)
                emit("sync", f)
                hw_xrt[t] = bump("hw_sp", 16)
                # PE: T1
                wait("tensor", "vd", vd_scale[t])
                if t - 2 >= 0:
                    wait("tensor", "ad", ad_cpA[t - 2])  # WAR tp1 psum
                def f(e, s=s):
                    e.matmul(tp1[s][:], lhsT=scaled[s][:], rhs=id_t[:],
                             is_transpose=True, start=True, stop=True).then_inc(pe, 1)
                emit("tensor", f)
                pe_t1[t] = bump("pe", 1)
                # ACT: copyA
                wait("scalar", "pe", pe_t1[t])
                if t - 2 >= 0:
                    wait("scalar", "pe", pe_mm[t - 2])   # WAR featT top half
                def f(e, s=s):
                    e.activation(featT[s][0:D, :], tp1[s][:], COPY).then_inc(ad, 1)
                emit("scalar", f)
                ad_cpA[t] = bump("ad", 1)
                # PE: MM (feat -> out)
                wait("tensor", "ad", ad_cpA[t])
                wait("tensor", "hw_sp", hw_xrt[t])
                if t - 2 >= 0:
                    wait("tensor", "vd", vd_bias[t - 2])  # WAR mmp psum
                def f(e, s=s):
                    e.matmul(mmp[s][:], lhsT=wt_t[:], rhs=featT[s][:],
                             start=True, stop=True).then_inc(pe, 1)
                emit("tensor", f)
                pe_mm[t] = bump("pe", 1)
                # DVE: bias add
                wait("vector", "pe", pe_mm[t])
                if t - 2 >= 0:
                    wait("vector", "hw_act", hw_out[t - 2])  # WAR outb slot
                def f(e, s=s):
                    e.tensor_add(out=outb[s][:], in0=mmp[s][:],
                                 in1=bias_t[:].to_broadcast([D, P])).then_inc(vd, 1)
                emit("vector", f)
                vd_bias[t] = bump("vd", 1)
                # ACT: store transposed output tile
                wait("scalar", "vd", vd_bias[t])
                def f(e, t=t, s=s):
                    e.dma_start(out=outT_d[:, t * P:(t + 1) * P],
                                in_=outb[s][:]).then_inc(hw_act, 16)
                emit("scalar", f)
                hw_out[t] = bump("hw_act", 16)

            # ---------- global order ----------
            for ci in range(NC0):
                emit_idx_dma(ci)
                emit_chunk_gathers(0, ci)
                emit_chunk_tree(0, ci)
            pre1 = min(2, NC1)
            for ci in range(NC0, NC0 + pre1):
                emit_idx_dma(ci)
                emit_chunk_gathers(1, ci)
                emit_chunk_tree(1, ci)
            emit_epilogue(0)
            for j in range(TB):
                emit_final_tile(0, j)
            for ci in range(NC0 + pre1, NC0 + NC1):
                emit_idx_dma(ci)
                emit_chunk_gathers(1, ci)
                emit_chunk_tree(1, ci)
            emit_epilogue(1)
            for j in range(TB):
                emit_final_tile(1, j)

            if debug:
                # dump S1 (via DRAM->DRAM copy) and agg tiles
                wait("sync", "sw", cnt["sw"])
                wait("sync", "vd", cnt["vd"])
                def fdbg(e):
                    e.dma_start(out=dbg_s1[0][:], in_=s1_d[0][:]).then_inc(hw_sp, 16)
                    e.dma_start(out=dbg_s1[1][:], in_=s1_d[1][:]).then_inc(hw_sp, 16)
                    e.dma_start(out=dbg_agg[0][:], in_=agg[0][:]).then_inc(hw_sp, 16)
                    e.dma_start(out=dbg_agg[1][:], in_=agg[1][:]).then_inc(hw_sp, 16)
                emit("sync", fdbg)
                bump("hw_sp", 64)

            @block.sync
            def _(eng):
                for f in S["sync"]:
                    f(eng)

            @block.gpsimd
            def _(eng):
                for f in S["gpsimd"]:
                    f(eng)

            @block.vector
            def _(eng):
                for f in S["vector"]:
                    f(eng)

            @block.scalar
            def _(eng):
                for f in S["scalar"]:
                    f(eng)

            @block.tensor
            def _(eng):
                for f in S["tensor"]:
                    f(eng)

    return nc


def kernel(x, row, colptr, W, b):
    global LAST_EXEC_TIME_NS, LAST_MEAN_EXEC_TIME_NS
    from concourse.bass_utils import run_bass_kernel_spmd

    x = np.asarray(x, np.float32)
    row = np.asarray(row, np.int32)
    colptr = np.asarray(colptr, np.int32)
    W = np.asarray(W, np.float32)
    b = np.asarray(b, np.float32)

    plan, in_maps = _plan_and_arrays(x, row, colptr, W, b)
    nc = _build(plan)

    # ensure the axon (neuron) backend is active even if the caller switched
    # jax to cpu (e.g. to run the reference)
    import jax
    if not any(d.platform != "cpu" for d in jax.devices()):
        jax.config.update("jax_platforms", "axon,cpu")
        from jax._src import xla_bridge
        xla_bridge._clear_backends()
        try:
            jax.clear_caches()
        except Exception:
            pass

    trace = bool(os.environ.get("BASS_KERNEL_TRACE"))
    kwargs = {}
    if trace:
        # the NTFF profile hook requires a live axon client connection
        import jax.numpy as jnp
        dev = [d for d in jax.devices() if d.platform != "cpu"][0]
        jax.jit(lambda a: a + 1)(jax.device_put(jnp.zeros((8,)), dev)).block_until_ready()
        kwargs = dict(trace=True, trace_cores=list(range(NCORES)))
    res = run_bass_kernel_spmd(nc, in_maps, list(range(NCORES)), **kwargs)
    LAST_EXEC_TIME_NS = res.exec_time_ns
    LAST_MEAN_EXEC_TIME_NS = res.mean_exec_time_ns
    globals()["LAST_RESULTS"] = res.results
    globals()["LAST_PLAN"] = plan
    globals()["LAST_IN_MAPS"] = in_maps

    out = np.empty((N, D), np.float32)
    bounds = plan["bounds"]
    for c in range(NCORES):
        n0, n1 = int(bounds[c]), int(bounds[c + 1])
        out[n0:n1] = res.results[c]["outT"].T[:n1 - n0]
    return out



# revision 8
# speedup vs baseline: 1.3201x; 1.3183x over previous
"""CuGraphSAGEConv (mean aggregation) on 8 trn2 NeuronCores via raw Bass.

Strategy
--------
Target nodes are sharded across 8 cores (edge-balanced CSC split); x is
replicated. Per core, each node's degree is decomposed exactly into parts
from a fixed class set (no wasted gather descriptors). Per (band, class):
one or two indirect-DMA gathers pull x[row[e]] rows into SBUF laid out as
[partition, group, K, 64]; a log-depth DVE fold reduces each K-group to a
partial node sum; partials are compacted and staged to DRAM (S1).
Multi-part nodes get a small second gather+fold ("combine") whose result
is written back to S1. A final gather collects one S1 row per node in
natural order; scale by 1/deg, PE-transpose, one matmul with W^T
(128->64), bias, transpose back, contiguous store.

The walrus build in this environment only accepts ONE attached sync-wait
per instruction, so everything is raw Bass with standalone wait_ge's.
Indirect-DMA facts established by probing the hardware:
  - idx tile [P, M] gathers row idx[p, m] into out[p, m*64:(m+1)*64]
    (p-major flat order); dest may be a partition/column slice.
  - out-of-bounds skipping only works for a tail suffix, so all padding
    indices here are in-bounds (row 0) and their results are discarded.
  - compute_op=add accumulates exactly.
"""

import os
import numpy as np

N = 100_000
E = 1_600_000
D = 64
NCORES = 8
P = 128

# exact degree decomposition classes (descending)
CLASSES = list(range(64, 32, -4)) + list(range(32, 0, -1))
SLOTS = 128          # gather-slot capacity per partition per chunk
GBUFS = 3            # chunk gather tile buffers
IBUFS = 3            # idx tile buffers
SPLIT_PIECE_MIN_K = 13  # classes >= this get a separate remainder gather

LAST_EXEC_TIME_NS = None
LAST_MEAN_EXEC_TIME_NS = None


def _decompose(d):
    parts = []
    while d > 64:
        parts.append(64)
        d -= 64
    if d > 32:
        b = d - (d % 4) if d >= 36 else 32
        parts.append(b)
        d -= b
    if d > 0:
        parts.append(d)
    return parts


def _fold_steps(w):
    steps = []
    while w > 1:
        h = 1 << ((w - 1).bit_length() - 1)
        if h == w:
            h = w // 2
        steps.append((h, w))
        w = h
    return steps


def _plan_and_arrays(x, row, colptr, W, b):
    deg = np.diff(colptr).astype(np.int64)

    cuts = np.searchsorted(colptr, (np.arange(1, NCORES) * (E // NCORES)).astype(colptr.dtype))
    bounds = np.concatenate([[0], cuts, [N]]).astype(np.int64)
    counts = np.diff(bounds)
    maxcount = int(counts.max())

    TB = (maxcount + 255) // 256
    band_nodes = TB * P
    ncap = 2 * band_nodes

    core_members = []
    core_nparts = []
    for c in range(NCORES):
        n0, n1 = int(bounds[c]), int(bounds[c + 1])
        cnt = n1 - n0
        dg = deg[n0:n1]
        npart = np.zeros(max(cnt, 1), np.int64)
        bands = [dict(), dict()]
        for loc in range(cnt):
            d = int(dg[loc])
            if d == 0:
                continue
            parts = _decompose(d)
            npart[loc] = len(parts)
            beta = 0 if loc < band_nodes else 1
            es = int(colptr[n0 + loc])
            for k, K in enumerate(parts):
                bands[beta].setdefault(K, []).append((loc, es, k))
                es += K
        core_members.append(bands)
        core_nparts.append(npart)

    PMAX = max(2, max(int(cn.max()) for cn in core_nparts))

    nmax = [dict(), dict()]
    for beta in range(2):
        for K in CLASSES:
            m = max(len(core_members[c][beta].get(K, [])) for c in range(NCORES))
            if m:
                nmax[beta][K] = m

    ncmb_max = [0, 0]
    for beta in range(2):
        for c in range(NCORES):
            npc = core_nparts[c]
            lo = beta * band_nodes
            hi = min(int(counts[c]), (beta + 1) * band_nodes)
            if hi > lo:
                ncmb_max[beta] = max(ncmb_max[beta], int((npc[lo:hi] >= 2).sum()))
    CM = [((ncmb_max[b] + P - 1) // P) if ncmb_max[b] else 0 for b in range(2)]

    band_layout = []
    for beta in range(2):
        pieces = []
        for K in CLASSES:
            n = nmax[beta].get(K, 0)
            if n == 0:
                continue
            if K >= SPLIT_PIECE_MIN_K:
                G = n // P
                r = n % P
                off = 0
                while G > 0:
                    g = min(G, SLOTS // K)
                    pieces.append(dict(K=K, ph=P, G=g, kind="bulk", mem_off=off))
                    off += P * g
                    G -= g
                if r:
                    pieces.append(dict(K=K, ph=r, G=1, kind="rem", mem_off=off))
            else:
                G1 = (n + P - 1) // P
                off = 0
                while G1 > 0:
                    g = min(G1, SLOTS // K)
                    pieces.append(dict(K=K, ph=P, G=g, kind="pad", mem_off=off))
                    off += P * g
                    G1 -= g
        chunk_cols = []
        for pc in pieces:
            w = pc["G"] * pc["K"]
            for ci, used in enumerate(chunk_cols):
                if used + w <= SLOTS:
                    pc["chunk"] = ci
                    pc["col0"] = used
                    chunk_cols[ci] = used + w
                    break
            else:
                pc["chunk"] = len(chunk_cols)
                pc["col0"] = 0
                chunk_cols.append(w)
        NC = len(chunk_cols)
        s1c = 1
        for pc in pieces:
            pc["s1c0"] = s1c
            s1c += pc["G"]
        cmb_col0 = s1c
        R = s1c + CM[beta]
        band_layout.append(dict(pieces=pieces, NC=NC, R=R, cmb_col0=cmb_col0))

    NC_tot = band_layout[0]["NC"] + band_layout[1]["NC"]

    row32 = row.astype(np.int32, copy=False)
    in_maps = []
    for c in range(NCORES):
        n0 = int(bounds[c])
        cnt = int(counts[c])
        idxf = np.zeros((NC_tot, P, SLOTS), np.int32)
        parts_rows = np.zeros((ncap, PMAX), np.int32)
        parts_cnt = np.zeros(ncap, np.int32)

        for beta in range(2):
            lay = band_layout[beta]
            R = lay["R"]
            chunk_base = 0 if beta == 0 else band_layout[0]["NC"]
            for K in CLASSES:
                if K not in nmax[beta]:
                    continue
                mem = core_members[c][beta].get(K, [])
                n = len(mem)
                if n:
                    nodes = np.array([m[0] for m in mem], np.int64)
                    es = np.array([m[1] for m in mem], np.int64)
                    ords = np.array([m[2] for m in mem], np.int64)
                    vals = row32[(es[:, None] + np.arange(K)).reshape(-1)].reshape(n, K)
                for pc in lay["pieces"]:
                    if pc["K"] != K:
                        continue
                    o = pc["mem_off"]
                    ncells = pc["ph"] * pc["G"]
                    a = np.zeros((ncells, K), np.int32)
                    take = max(0, min(n - o, ncells)) if n else 0
                    if take:
                        a[:take] = vals[o:o + take]
                    ci = chunk_base + pc["chunk"]
                    s0 = pc["col0"]
                    if pc["kind"] == "rem":
                        idxf[ci, :pc["ph"], s0:s0 + K] = a
                    else:
                        idxf[ci, :, s0:s0 + pc["G"] * K] = a.reshape(P, pc["G"] * K)
                    if take:
                        j = np.arange(take)
                        if pc["kind"] == "rem":
                            rows = j * R + pc["s1c0"]
                        else:
                            rows = (j // pc["G"]) * R + pc["s1c0"] + (j % pc["G"])
                        nd = nodes[o:o + take]
                        od = ords[o:o + take]
                        parts_rows[nd, od] = rows.astype(np.int32)
                        parts_cnt[nd] = np.maximum(parts_cnt[nd], (od + 1).astype(np.int32))

        cmbi = []
        l2i = np.zeros((2, P, TB), np.int32)
        for beta in range(2):
            lay = band_layout[beta]
            R = lay["R"]
            cmw = max(1, CM[beta] * PMAX)
            ci_arr = np.zeros((P, cmw), np.int32)
            lo = beta * band_nodes
            hi = min(cnt, (beta + 1) * band_nodes)
            node_row = np.zeros(band_nodes, np.int32)
            if hi > lo:
                loc = np.arange(lo, hi)
                pcn = parts_cnt[lo:hi]
                single = pcn == 1
                node_row[:hi - lo][single] = parts_rows[lo:hi][single, 0]
                multi_nodes = loc[pcn >= 2]
                for m, nd in enumerate(multi_nodes):
                    p_ = m // CM[beta]
                    jm = m % CM[beta]
                    for k in range(int(parts_cnt[nd])):
                        ci_arr[p_, jm * PMAX + k] = parts_rows[nd, k]
                    node_row[nd - lo] = p_ * R + lay["cmb_col0"] + jm
            cmbi.append(ci_arr)
            l2i[beta] = node_row.reshape(TB, P).T

        recip = np.ones(ncap, np.float32)
        recip[:cnt] = 1.0 / np.maximum(deg[n0:n0 + cnt].astype(np.float32), 1.0)
        recip_pk = recip.reshape(2, TB, P).transpose(0, 2, 1).copy()
        xroot = np.zeros((ncap, D), np.float32)
        xroot[:cnt] = x[n0:n0 + cnt]

        in_maps.append({
            "x": np.ascontiguousarray(x),
            "idxf": idxf,
            "cmbi0": cmbi[0], "cmbi1": cmbi[1],
            "l2i": l2i,
            "recip": recip_pk,
            "xrootT": np.ascontiguousarray(xroot.T),
            "Wt": np.ascontiguousarray(W.T.astype(np.float32)),
            "bias": b.astype(np.float32).reshape(D, 1),
            "ident": np.eye(P, dtype=np.float32),
        })

    plan = dict(TB=TB, band_nodes=band_nodes, ncap=ncap, PMAX=PMAX, CM=CM,
                band_layout=band_layout, NC_tot=NC_tot, bounds=bounds, counts=counts)
    return plan, in_maps


def _build(plan):
    import concourse.bass as bass
    import concourse.mybir as mybir
    from contextlib import ExitStack

    f32 = mybir.dt.float32
    i32 = mybir.dt.int32
    COPY = mybir.ActivationFunctionType.Copy
    TB = plan["TB"]
    PMAX = plan["PMAX"]
    CM = plan["CM"]
    lay = plan["band_layout"]
    NC0 = lay[0]["NC"]
    NC1 = lay[1]["NC"]
    NC_tot = plan["NC_tot"]
    ncap = plan["ncap"]

    nc = bass.Bass()
    x_d = nc.declare_dram_parameter("x", [N, D], f32, isOutput=False)
    idxf_d = nc.declare_dram_parameter("idxf", [NC_tot, P, SLOTS], i32, isOutput=False)
    cmb_d = [nc.declare_dram_parameter(f"cmbi{b2}", [P, max(1, CM[b2] * PMAX)], i32,
                                       isOutput=False) for b2 in range(2)]
    l2i_d = nc.declare_dram_parameter("l2i", [2, P, TB], i32, isOutput=False)
    recip_d = nc.declare_dram_parameter("recip", [2, P, TB], f32, isOutput=False)
    xrootT_d = nc.declare_dram_parameter("xrootT", [D, ncap], f32, isOutput=False)
    wt_d = nc.declare_dram_parameter("Wt", [P, D], f32, isOutput=False)
    b_d = nc.declare_dram_parameter("bias", [D, 1], f32, isOutput=False)
    id_d = nc.declare_dram_parameter("ident", [P, P], f32, isOutput=False)
    outT_d = nc.declare_dram_parameter("outT", [D, ncap], f32, isOutput=True)
    debug = bool(os.environ.get("BASS_KERNEL_DEBUG"))
    if debug:
        dbg_s1 = [nc.declare_dram_parameter(f"dbg_s1_{b2}", [P * lay[b2]["R"], D], f32,
                                            isOutput=True) for b2 in range(2)]
        dbg_agg = [nc.declare_dram_parameter(f"dbg_agg{b2}", [P, TB * D], f32,
                                             isOutput=True) for b2 in range(2)]

    s1_d = [nc.dram_tensor(f"s1_{b2}", [P * lay[b2]["R"], D], f32) for b2 in range(2)]

    ctx = ExitStack()
    sb = lambda nm, shape, dt: ctx.enter_context(nc.sbuf_tensor(nm, shape, dt))
    ps = lambda nm, shape: ctx.enter_context(nc.psum_tensor(nm, shape, f32))

    with ctx:
        g_t = [sb(f"g{i}", [P, SLOTS * D], f32) for i in range(GBUFS)]
        idx_t = [sb(f"ix{i}", [P, SLOTS], i32) for i in range(IBUFS)]
        s1sb = [sb(f"s1sb{i}", [P, lay[i]["R"] * D], f32) for i in range(2)]
        agg = [sb(f"agg{i}", [P, TB * D], f32) for i in range(2)]
        cmbg = [sb(f"cmbg{i}", [P, max(1, CM[i] * PMAX) * D], f32) for i in range(2)]
        cmbi_t = [sb(f"cmbit{i}", [P, max(1, CM[i] * PMAX)], i32) for i in range(2)]
        l2i_t = [sb(f"l2it{i}", [P, TB], i32) for i in range(2)]
        recip_t = [sb(f"recipt{i}", [P, TB], f32) for i in range(2)]
        wt_t = sb("wtt", [P, D], f32)
        bias_t = sb("biast", [D, 1], f32)
        id_t = sb("idt", [P, P], f32)
        scaled = [sb(f"scaled{i}", [P, D], f32) for i in range(2)]
        featT = [sb(f"featT{i}", [P, P], f32) for i in range(2)]
        outb = [sb(f"outb{i}", [D, P], f32) for i in range(4)]
        tp1 = [ps(f"tp1_{i}", [D, P]) for i in range(2)]
        mmp = [ps(f"mmp{i}", [D, P]) for i in range(2)]

        with (
            nc.semaphore("hw_sp") as hw_sp,
            nc.semaphore("hw_act") as hw_act,
            nc.semaphore("sw") as sw,
            nc.semaphore("vd") as vd,
            nc.semaphore("ad") as ad,
            nc.semaphore("pe") as pe,
            nc.Block() as block,
        ):
            cnt = dict(hw_sp=0, hw_act=0, sw=0, vd=0, ad=0, pe=0)
            S = dict(sync=[], gpsimd=[], vector=[], scalar=[], tensor=[])
            sem_obj = dict(hw_sp=hw_sp, hw_act=hw_act, sw=sw, vd=vd, ad=ad, pe=pe)

            def emit(eng, fn):
                S[eng].append(fn)

            def wait(eng, sem, val):
                if val > 0:
                    S[eng].append(lambda e, s=sem_obj[sem], v=val: e.wait_ge(s, v))

            def bump(sem, by):
                cnt[sem] += by
                return cnt[sem]

            # ---------- prelude ----------
            def prelude(e):
                e.dma_start(out=wt_t[:], in_=wt_d[:]).then_inc(hw_sp, 16)
                e.dma_start(out=bias_t[:], in_=b_d[:]).then_inc(hw_sp, 16)
                e.dma_start(out=id_t[:], in_=id_d[:]).then_inc(hw_sp, 16)
                e.dma_start(out=recip_t[0][:], in_=recip_d[0]).then_inc(hw_sp, 16)
                e.dma_start(out=recip_t[1][:], in_=recip_d[1]).then_inc(hw_sp, 16)
                e.dma_start(out=l2i_t[0][:], in_=l2i_d[0]).then_inc(hw_sp, 16)
                e.dma_start(out=l2i_t[1][:], in_=l2i_d[1]).then_inc(hw_sp, 16)
            emit("sync", prelude)
            hw_sp_prelude = bump("hw_sp", 7 * 16)

            wait("vector", "hw_sp", hw_sp_prelude)
            def zerocols(e):
                e.memset(s1sb[0][:, 0:D], 0.0)
                e.memset(s1sb[1][:, 0:D], 0.0).then_inc(vd, 1)
            emit("vector", zerocols)
            vd_zero = bump("vd", 1)

            wait("tensor", "hw_sp", hw_sp_prelude)

            # ---------- L1 ----------
            hw_after_idx = {}
            sw_after_chunk = {}
            vd_after_chunk = {}

            def chunk_pieces(beta, ci):
                local = ci - (0 if beta == 0 else NC0)
                return [pc for pc in lay[beta]["pieces"] if pc["chunk"] == local]

            def emit_idx_dma(ci):
                slot = ci % IBUFS
                prev = ci - IBUFS
                if prev >= 0:
                    wait("sync", "sw", sw_after_chunk[prev])
                def f(e, ci=ci, slot=slot):
                    e.dma_start(out=idx_t[slot][:], in_=idxf_d[ci]).then_inc(hw_sp, 16)
                emit("sync", f)
                hw_after_idx[ci] = bump("hw_sp", 16)

            def emit_chunk_gathers(beta, ci):
                slot = ci % IBUFS
                gslot = ci % GBUFS
                wait("gpsimd", "hw_sp", hw_after_idx[ci])
                prevg = ci - GBUFS
                if prevg >= 0:
                    wait("gpsimd", "vd", vd_after_chunk[prevg])
                # HW quirk: the indirect DMA consumes exactly ONE index per
                # partition -> one gather instruction per slot column.
                for pc in chunk_pieces(beta, ci):
                    s0, Wd = pc["col0"], pc["G"] * pc["K"]
                    ph = max(pc["ph"], 2)  # 1-row indirect DMAs unsupported
                    for c in range(s0, s0 + Wd):
                        def f(e, c=c, ph=ph, slot=slot, gslot=gslot):
                            e.indirect_dma_start(
                                out=g_t[gslot][0:ph, c * D:(c + 1) * D],
                                out_offset=None,
                                in_=x_d[:],
                                in_offset=bass.IndirectOffsetOnAxis(
                                    ap=idx_t[slot][0:ph, c:c + 1], axis=0),
                            ).then_inc(sw, 16)
                        emit("gpsimd", f)
                        bump("sw", 16)
                sw_after_chunk[ci] = cnt["sw"]

            def emit_chunk_tree(beta, ci):
                gslot = ci % GBUFS
                wait("vector", "sw", sw_after_chunk[ci])
                pcs = chunk_pieces(beta, ci)
                for pi, pc in enumerate(pcs):
                    K, G, ph, s0, c0 = pc["K"], pc["G"], pc["ph"], pc["col0"], pc["s1c0"]
                    base = g_t[gslot]
                    for (h, w) in _fold_steps(K):
                        def f(e, base=base, ph=ph, G=G, K=K, s0=s0, h=h, w=w):
                            v = base[0:ph, s0 * D:(s0 + G * K) * D].rearrange(
                                "p (g k f) -> p g k f", g=G, f=D)
                            e.tensor_add(out=v[:, :, 0:w - h, :],
                                         in0=v[:, :, 0:w - h, :],
                                         in1=v[:, :, h:w, :])
                        emit("vector", f)
                    is_last = pi == len(pcs) - 1
                    def f(e, base=base, ph=ph, G=G, K=K, s0=s0, c0=c0,
                          beta=beta, is_last=is_last):
                        src = base[0:ph, s0 * D:(s0 + G * K) * D].rearrange(
                            "p (g k f) -> p g k f", g=G, f=D)[:, :, 0, :]
                        dst = s1sb[beta][0:ph, c0 * D:(c0 + G) * D].rearrange(
                            "p (g f) -> p g f", g=G)
                        ins = e.tensor_copy(out=dst, in_=src)
                        if is_last:
                            ins.then_inc(vd, 1)
                    emit("vector", f)
                vd_after_chunk[ci] = bump("vd", 1)

            # ---------- band epilogue ----------
            sw_after_l2 = {}

            def emit_epilogue(beta):
                R = lay[beta]["R"]
                last_ci = (NC0 - 1) if beta == 0 else (NC0 + NC1 - 1)
                cc0 = lay[beta]["cmb_col0"]
                wait("sync", "vd", max(vd_after_chunk[last_ci], vd_zero))
                def f(e, beta=beta, R=R, cc0=cc0):
                    dst = s1_d[beta][:].rearrange("(p r) f -> p r f", r=R)[:, 0:cc0, :]
                    src = s1sb[beta][:, 0:cc0 * D].rearrange("p (r f) -> p r f", f=D)
                    e.dma_start(out=dst, in_=src).then_inc(hw_sp, 16)
                emit("sync", f)
                hw_s1w = bump("hw_sp", 16)
                if CM[beta]:
                    def f(e, beta=beta):
                        e.dma_start(out=cmbi_t[beta][:], in_=cmb_d[beta][:]).then_inc(hw_sp, 16)
                    emit("sync", f)
                    hw_cmbi = bump("hw_sp", 16)
                    wait("gpsimd", "hw_sp", hw_cmbi)
                    for c in range(CM[beta] * PMAX):
                        def f(e, beta=beta, c=c):
                            e.indirect_dma_start(
                                out=cmbg[beta][:, c * D:(c + 1) * D],
                                out_offset=None,
                                in_=s1_d[beta][:],
                                in_offset=bass.IndirectOffsetOnAxis(
                                    ap=cmbi_t[beta][:, c:c + 1], axis=0),
                            ).then_inc(sw, 16)
                        emit("gpsimd", f)
                        bump("sw", 16)
                    sw_cmbg = cnt["sw"]
                    wait("vector", "sw", sw_cmbg)
                    for (h, w) in _fold_steps(PMAX):
                        def f(e, beta=beta, h=h, w=w):
                            v = cmbg[beta][:, 0:CM[beta] * PMAX * D].rearrange(
                                "p (m k f) -> p m k f", m=CM[beta], f=D)
                            e.tensor_add(out=v[:, :, 0:w - h, :],
                                         in0=v[:, :, 0:w - h, :],
                                         in1=v[:, :, h:w, :])
                        emit("vector", f)
                    def f(e, beta=beta, cc0=cc0):
                        src = cmbg[beta][:, 0:CM[beta] * PMAX * D].rearrange(
                            "p (m k f) -> p m k f", m=CM[beta], f=D)[:, :, 0, :]
                        dst = s1sb[beta][:, cc0 * D:(cc0 + CM[beta]) * D].rearrange(
                            "p (m f) -> p m f", f=D)
                        e.tensor_copy(out=dst, in_=src).then_inc(vd, 1)
                    emit("vector", f)
                    vd_cmbf = bump("vd", 1)
                    wait("sync", "vd", vd_cmbf)
                    def f(e, beta=beta, R=R, cc0=cc0):
                        dst = s1_d[beta][:].rearrange("(p r) f -> p r f", r=R)[:, cc0:cc0 + CM[beta], :]
                        src = s1sb[beta][:, cc0 * D:(cc0 + CM[beta]) * D].rearrange(
                            "p (m f) -> p m f", f=D)
                        e.dma_start(out=dst, in_=src).then_inc(hw_sp, 16)
                    emit("sync", f)
                    hw_s1w2 = bump("hw_sp", 16)
                    wait("gpsimd", "hw_sp", hw_s1w2)
                else:
                    wait("gpsimd", "hw_sp", hw_s1w)
                for j in range(TB):
                    def f(e, beta=beta, j=j):
                        e.indirect_dma_start(
                            out=agg[beta][:, j * D:(j + 1) * D],
                            out_offset=None,
                            in_=s1_d[beta][:],
                            in_offset=bass.IndirectOffsetOnAxis(
                                ap=l2i_t[beta][:, j:j + 1], axis=0),
                        ).then_inc(sw, 16)
                    emit("gpsimd", f)
                    bump("sw", 16)
                sw_after_l2[beta] = cnt["sw"]

            # ---------- final phase ----------
            # Per tile: DVE scale -> PE transpose(agg) -> ACT copy to featT
            # top half; xrootT tile DMA'd (host-pretransposed) into featT
            # bottom half; PE matmul W^T @ featT; DVE bias; store the [D, P]
            # result TRANSPOSED to outT (host un-transposes after readback).
            pe_t1, pe_mm = {}, {}
            ad_cpA = {}
            hw_xrt, hw_out = {}, {}
            vd_scale, vd_bias = {}, {}

            def emit_final_tile(beta, j):
                t = beta * TB + j
                s = t % 2
                # DVE: scale by 1/deg
                if j == 0:
                    wait("vector", "sw", sw_after_l2[beta])
                if t - 2 >= 0:
                    wait("vector", "pe", pe_t1[t - 2])   # WAR scaled slot
                def f(e, beta=beta, j=j, s=s):
                    e.tensor_scalar_mul(scaled[s][:], agg[beta][:, j * D:(j + 1) * D],
                                        recip_t[beta][:, j:j + 1]).then_inc(vd, 1)
                emit("vector", f)
                vd_scale[t] = bump("vd", 1)
                # SYNC: xrootT tile load into featT bottom half
                if t - 2 >= 0:
                    wait("sync", "pe", pe_mm[t - 2])     # WAR featT slot
                def f(e, t=t, s=s):
                    e.dma_start(out=featT[s][D:P, :],
                                in_=xrootT_d[:, t * P:(t + 1) * P]).then_inc(hw_sp, 16)
                emit("sync", f)
                hw_xrt[t] = bump("hw_sp", 16)
                # PE: T1
                wait("tensor", "vd", vd_scale[t])
                if t - 2 >= 0:
                    wait("tensor", "ad", ad_cpA[t - 2])  # WAR tp1 psum
                def f(e, s=s):
                    e.matmul(tp1[s][:], lhsT=scaled[s][:], rhs=id_t[:],
                             is_transpose=True, start=True, stop=True).then_inc(pe, 1)
                emit("tensor", f)
                pe_t1[t] = bump("pe", 1)
                # ACT: copyA
                wait("scalar", "pe", pe_t1[t])
                if t - 2 >= 0:
                    wait("scalar", "pe", pe_mm[t - 2])   # WAR featT top half
                def f(e, s=s):
                    e.activation(featT[s][0:D, :], tp1[s][:], COPY).then_inc(ad, 1)
                emit("scalar", f)
                ad_cpA[t] = bump("ad", 1)
                # PE: MM (feat -> out)
                wait("tensor", "ad", ad_cpA[t])
                wait("tensor", "hw_sp", hw_xrt[t])
                if t - 2 >= 0:
                    wait("tensor", "vd", vd_bias[t - 2])  # WAR mmp psum
                def f(e, s=s):
                    e.matmul(mmp[s][:], lhsT=wt_t[:], rhs=featT[s][:],
                             start=True, stop=True).then_inc(pe, 1)
                emit("tensor", f)
                pe_mm[t] = bump("pe", 1)
                # DVE: bias add
                s4 = t % 4
                wait("vector", "pe", pe_mm[t])
                if t - 4 >= 0:
                    wait("vector", "hw_act", hw_out[t - 4])  # WAR outb slot
                def f(e, s=s, s4=s4):
                    e.tensor_add(out=outb[s4][:], in0=mmp[s][:],
                                 in1=bias_t[:].to_broadcast([D, P])).then_inc(vd, 1)
                emit("vector", f)
                vd_bias[t] = bump("vd", 1)
                # ACT: store transposed output tile
                wait("scalar", "vd", vd_bias[t])
                def f(e, t=t, s4=s4):
                    e.dma_start(out=outT_d[:, t * P:(t + 1) * P],
                                in_=outb[s4][:]).then_inc(hw_act, 16)
                emit("scalar", f)
                hw_out[t] = bump("hw_act", 16)

            # ---------- global order ----------
            for ci in range(NC0):
                emit_idx_dma(ci)
                emit_chunk_gathers(0, ci)
                emit_chunk_tree(0, ci)
            pre1 = min(2, NC1)
            for ci in range(NC0, NC0 + pre1):
                emit_idx_dma(ci)
                emit_chunk_gathers(1, ci)
                emit_chunk_tree(1, ci)
            emit_epilogue(0)
            for j in range(TB):
                emit_final_tile(0, j)
            for ci in range(NC0 + pre1, NC0 + NC1):
                emit_idx_dma(ci)
                emit_chunk_gathers(1, ci)
                emit_chunk_tree(1, ci)
            emit_epilogue(1)
            for j in range(TB):
                emit_final_tile(1, j)

            if debug:
                # dump S1 (via DRAM->DRAM copy) and agg tiles
                wait("sync", "sw", cnt["sw"])
                wait("sync", "vd", cnt["vd"])
                def fdbg(e):
                    e.dma_start(out=dbg_s1[0][:], in_=s1_d[0][:]).then_inc(hw_sp, 16)
                    e.dma_start(out=dbg_s1[1][:], in_=s1_d[1][:]).then_inc(hw_sp, 16)
                    e.dma_start(out=dbg_agg[0][:], in_=agg[0][:]).then_inc(hw_sp, 16)
                    e.dma_start(out=dbg_agg[1][:], in_=agg[1][:]).then_inc(hw_sp, 16)
                emit("sync", fdbg)
                bump("hw_sp", 64)

            @block.sync
            def _(eng):
                for f in S["sync"]:
                    f(eng)

            @block.gpsimd
            def _(eng):
                for f in S["gpsimd"]:
                    f(eng)

            @block.vector
            def _(eng):
                for f in S["vector"]:
                    f(eng)

            @block.scalar
            def _(eng):
                for f in S["scalar"]:
                    f(eng)

            @block.tensor
            def _(eng):
                for f in S["tensor"]:
                    f(eng)

    return nc


def kernel(x, row, colptr, W, b):
    global LAST_EXEC_TIME_NS, LAST_MEAN_EXEC_TIME_NS
    from concourse.bass_utils import run_bass_kernel_spmd

    x = np.asarray(x, np.float32)
    row = np.asarray(row, np.int32)
    colptr = np.asarray(colptr, np.int32)
    W = np.asarray(W, np.float32)
    b = np.asarray(b, np.float32)

    plan, in_maps = _plan_and_arrays(x, row, colptr, W, b)
    nc = _build(plan)

    # ensure the axon (neuron) backend is active even if the caller switched
    # jax to cpu (e.g. to run the reference)
    import jax
    if not any(d.platform != "cpu" for d in jax.devices()):
        jax.config.update("jax_platforms", "axon,cpu")
        from jax._src import xla_bridge
        xla_bridge._clear_backends()
        try:
            jax.clear_caches()
        except Exception:
            pass

    trace = bool(os.environ.get("BASS_KERNEL_TRACE"))
    kwargs = {}
    if trace:
        # the NTFF profile hook requires a live axon client connection
        import jax.numpy as jnp
        dev = [d for d in jax.devices() if d.platform != "cpu"][0]
        jax.jit(lambda a: a + 1)(jax.device_put(jnp.zeros((8,)), dev)).block_until_ready()
        kwargs = dict(trace=True, trace_cores=list(range(NCORES)))
    res = run_bass_kernel_spmd(nc, in_maps, list(range(NCORES)), **kwargs)
    LAST_EXEC_TIME_NS = res.exec_time_ns
    LAST_MEAN_EXEC_TIME_NS = res.mean_exec_time_ns
    globals()["LAST_RESULTS"] = res.results
    globals()["LAST_PLAN"] = plan
    globals()["LAST_IN_MAPS"] = in_maps

    out = np.empty((N, D), np.float32)
    bounds = plan["bounds"]
    for c in range(NCORES):
        n0, n1 = int(bounds[c]), int(bounds[c + 1])
        out[n0:n1] = res.results[c]["outT"].T[:n1 - n0]
    return out



# revision 11
# speedup vs baseline: 1.3222x; 1.0016x over previous
"""CuGraphSAGEConv (mean aggregation) on 8 trn2 NeuronCores via raw Bass.

Strategy
--------
Target nodes are sharded across 8 cores (edge-balanced CSC split); x is
replicated. Per core, each node's degree is decomposed exactly into parts
from a fixed class set (no wasted gather descriptors). Per (band, class):
one or two indirect-DMA gathers pull x[row[e]] rows into SBUF laid out as
[partition, group, K, 64]; a log-depth DVE fold reduces each K-group to a
partial node sum; partials are compacted and staged to DRAM (S1).
Multi-part nodes get a small second gather+fold ("combine") whose result
is written back to S1. A final gather collects one S1 row per node in
natural order; scale by 1/deg, PE-transpose, one matmul with W^T
(128->64), bias, transpose back, contiguous store.

The walrus build in this environment only accepts ONE attached sync-wait
per instruction, so everything is raw Bass with standalone wait_ge's.
Indirect-DMA facts established by probing the hardware:
  - idx tile [P, M] gathers row idx[p, m] into out[p, m*64:(m+1)*64]
    (p-major flat order); dest may be a partition/column slice.
  - out-of-bounds skipping only works for a tail suffix, so all padding
    indices here are in-bounds (row 0) and their results are discarded.
  - compute_op=add accumulates exactly.
"""

import os
import numpy as np

N = 100_000
E = 1_600_000
D = 64
NCORES = 8
P = 128

# exact degree decomposition classes (descending)
CLASSES = list(range(64, 32, -4)) + list(range(32, 0, -1))
SLOTS = 128          # gather-slot capacity per partition per chunk
GBUFS = 3            # chunk gather tile buffers
IBUFS = 3            # idx tile buffers
SPLIT_PIECE_MIN_K = 13  # classes >= this get a separate remainder gather

LAST_EXEC_TIME_NS = None
LAST_MEAN_EXEC_TIME_NS = None


def _decompose(d):
    parts = []
    while d > 64:
        parts.append(64)
        d -= 64
    if d > 32:
        b = d - (d % 4) if d >= 36 else 32
        parts.append(b)
        d -= b
    if d > 0:
        parts.append(d)
    return parts


def _fold_steps(w):
    steps = []
    while w > 1:
        h = 1 << ((w - 1).bit_length() - 1)
        if h == w:
            h = w // 2
        steps.append((h, w))
        w = h
    return steps


def _plan_and_arrays(x, row, colptr, W, b):
    deg = np.diff(colptr).astype(np.int64)

    cuts = np.searchsorted(colptr, (np.arange(1, NCORES) * (E // NCORES)).astype(colptr.dtype))
    bounds = np.concatenate([[0], cuts, [N]]).astype(np.int64)
    counts = np.diff(bounds)
    maxcount = int(counts.max())

    TB = (maxcount + 255) // 256
    band_nodes = TB * P
    ncap = 2 * band_nodes

    core_members = []
    core_nparts = []
    for c in range(NCORES):
        n0, n1 = int(bounds[c]), int(bounds[c + 1])
        cnt = n1 - n0
        dg = deg[n0:n1]
        npart = np.zeros(max(cnt, 1), np.int64)
        bands = [dict(), dict()]
        for loc in range(cnt):
            d = int(dg[loc])
            if d == 0:
                continue
            parts = _decompose(d)
            npart[loc] = len(parts)
            beta = 0 if loc < band_nodes else 1
            es = int(colptr[n0 + loc])
            for k, K in enumerate(parts):
                bands[beta].setdefault(K, []).append((loc, es, k))
                es += K
        core_members.append(bands)
        core_nparts.append(npart)

    PMAX = max(2, max(int(cn.max()) for cn in core_nparts))

    nmax = [dict(), dict()]
    for beta in range(2):
        for K in CLASSES:
            m = max(len(core_members[c][beta].get(K, [])) for c in range(NCORES))
            if m:
                nmax[beta][K] = m

    ncmb_max = [0, 0]
    for beta in range(2):
        for c in range(NCORES):
            npc = core_nparts[c]
            lo = beta * band_nodes
            hi = min(int(counts[c]), (beta + 1) * band_nodes)
            if hi > lo:
                ncmb_max[beta] = max(ncmb_max[beta], int((npc[lo:hi] >= 2).sum()))
    CM = [((ncmb_max[b] + P - 1) // P) if ncmb_max[b] else 0 for b in range(2)]

    band_layout = []
    for beta in range(2):
        pieces = []
        for K in CLASSES:
            n = nmax[beta].get(K, 0)
            if n == 0:
                continue
            if K >= SPLIT_PIECE_MIN_K:
                G = n // P
                r = n % P
                off = 0
                while G > 0:
                    g = min(G, SLOTS // K)
                    pieces.append(dict(K=K, ph=P, G=g, kind="bulk", mem_off=off))
                    off += P * g
                    G -= g
                if r:
                    pieces.append(dict(K=K, ph=r, G=1, kind="rem", mem_off=off))
            else:
                G1 = (n + P - 1) // P
                off = 0
                while G1 > 0:
                    g = min(G1, SLOTS // K)
                    pieces.append(dict(K=K, ph=P, G=g, kind="pad", mem_off=off))
                    off += P * g
                    G1 -= g
        chunk_cols = []
        for pc in pieces:
            w = pc["G"] * pc["K"]
            for ci, used in enumerate(chunk_cols):
                if used + w <= SLOTS:
                    pc["chunk"] = ci
                    pc["col0"] = used
                    chunk_cols[ci] = used + w
                    break
            else:
                pc["chunk"] = len(chunk_cols)
                pc["col0"] = 0
                chunk_cols.append(w)
        NC = len(chunk_cols)
        s1c = 1
        for pc in pieces:
            pc["s1c0"] = s1c
            s1c += pc["G"]
        cmb_col0 = s1c
        R = s1c + CM[beta]
        band_layout.append(dict(pieces=pieces, NC=NC, R=R, cmb_col0=cmb_col0))

    NC_tot = band_layout[0]["NC"] + band_layout[1]["NC"]

    row32 = row.astype(np.int32, copy=False)
    in_maps = []
    for c in range(NCORES):
        n0 = int(bounds[c])
        cnt = int(counts[c])
        idxf = np.zeros((NC_tot, P, SLOTS), np.int32)
        parts_rows = np.zeros((ncap, PMAX), np.int32)
        parts_cnt = np.zeros(ncap, np.int32)

        for beta in range(2):
            lay = band_layout[beta]
            R = lay["R"]
            chunk_base = 0 if beta == 0 else band_layout[0]["NC"]
            for K in CLASSES:
                if K not in nmax[beta]:
                    continue
                mem = core_members[c][beta].get(K, [])
                n = len(mem)
                if n:
                    nodes = np.array([m[0] for m in mem], np.int64)
                    es = np.array([m[1] for m in mem], np.int64)
                    ords = np.array([m[2] for m in mem], np.int64)
                    vals = row32[(es[:, None] + np.arange(K)).reshape(-1)].reshape(n, K)
                for pc in lay["pieces"]:
                    if pc["K"] != K:
                        continue
                    o = pc["mem_off"]
                    ncells = pc["ph"] * pc["G"]
                    a = np.zeros((ncells, K), np.int32)
                    take = max(0, min(n - o, ncells)) if n else 0
                    if take:
                        a[:take] = vals[o:o + take]
                    ci = chunk_base + pc["chunk"]
                    s0 = pc["col0"]
                    if pc["kind"] == "rem":
                        idxf[ci, :pc["ph"], s0:s0 + K] = a
                    else:
                        idxf[ci, :, s0:s0 + pc["G"] * K] = a.reshape(P, pc["G"] * K)
                    if take:
                        j = np.arange(take)
                        if pc["kind"] == "rem":
                            rows = j * R + pc["s1c0"]
                        else:
                            rows = (j // pc["G"]) * R + pc["s1c0"] + (j % pc["G"])
                        nd = nodes[o:o + take]
                        od = ords[o:o + take]
                        parts_rows[nd, od] = rows.astype(np.int32)
                        parts_cnt[nd] = np.maximum(parts_cnt[nd], (od + 1).astype(np.int32))

        cmbi = []
        l2i = np.zeros((2, P, TB), np.int32)
        for beta in range(2):
            lay = band_layout[beta]
            R = lay["R"]
            cmw = max(1, CM[beta] * PMAX)
            ci_arr = np.zeros((P, cmw), np.int32)
            lo = beta * band_nodes
            hi = min(cnt, (beta + 1) * band_nodes)
            node_row = np.zeros(band_nodes, np.int32)
            if hi > lo:
                loc = np.arange(lo, hi)
                pcn = parts_cnt[lo:hi]
                single = pcn == 1
                node_row[:hi - lo][single] = parts_rows[lo:hi][single, 0]
                multi_nodes = loc[pcn >= 2]
                for m, nd in enumerate(multi_nodes):
                    p_ = m // CM[beta]
                    jm = m % CM[beta]
                    for k in range(int(parts_cnt[nd])):
                        ci_arr[p_, jm * PMAX + k] = parts_rows[nd, k]
                    node_row[nd - lo] = p_ * R + lay["cmb_col0"] + jm
            cmbi.append(ci_arr)
            l2i[beta] = node_row.reshape(TB, P).T

        recip = np.ones(ncap, np.float32)
        recip[:cnt] = 1.0 / np.maximum(deg[n0:n0 + cnt].astype(np.float32), 1.0)
        recip_pk = recip.reshape(2, TB, P).transpose(0, 2, 1).copy()
        xroot = np.zeros((ncap, D), np.float32)
        xroot[:cnt] = x[n0:n0 + cnt]

        in_maps.append({
            "x": np.ascontiguousarray(x),
            "idxf": idxf,
            "cmbi0": cmbi[0], "cmbi1": cmbi[1],
            "l2i": l2i,
            "recip": recip_pk,
            "xrootT": np.ascontiguousarray(xroot.T),
            "Wt": np.ascontiguousarray(W.T.astype(np.float32)),
            "bias": b.astype(np.float32).reshape(D, 1),
            "ident": np.eye(P, dtype=np.float32),
        })

    plan = dict(TB=TB, band_nodes=band_nodes, ncap=ncap, PMAX=PMAX, CM=CM,
                band_layout=band_layout, NC_tot=NC_tot, bounds=bounds, counts=counts)
    return plan, in_maps


def _build(plan):
    import concourse.bass as bass
    import concourse.mybir as mybir
    from contextlib import ExitStack

    f32 = mybir.dt.float32
    i32 = mybir.dt.int32
    COPY = mybir.ActivationFunctionType.Copy
    TB = plan["TB"]
    PMAX = plan["PMAX"]
    CM = plan["CM"]
    lay = plan["band_layout"]
    NC0 = lay[0]["NC"]
    NC1 = lay[1]["NC"]
    NC_tot = plan["NC_tot"]
    ncap = plan["ncap"]

    nc = bass.Bass()
    x_d = nc.declare_dram_parameter("x", [N, D], f32, isOutput=False)
    idxf_d = nc.declare_dram_parameter("idxf", [NC_tot, P, SLOTS], i32, isOutput=False)
    cmb_d = [nc.declare_dram_parameter(f"cmbi{b2}", [P, max(1, CM[b2] * PMAX)], i32,
                                       isOutput=False) for b2 in range(2)]
    l2i_d = nc.declare_dram_parameter("l2i", [2, P, TB], i32, isOutput=False)
    recip_d = nc.declare_dram_parameter("recip", [2, P, TB], f32, isOutput=False)
    xrootT_d = nc.declare_dram_parameter("xrootT", [D, ncap], f32, isOutput=False)
    wt_d = nc.declare_dram_parameter("Wt", [P, D], f32, isOutput=False)
    b_d = nc.declare_dram_parameter("bias", [D, 1], f32, isOutput=False)
    id_d = nc.declare_dram_parameter("ident", [P, P], f32, isOutput=False)
    outT_d = nc.declare_dram_parameter("outT", [D, ncap], f32, isOutput=True)
    debug = bool(os.environ.get("BASS_KERNEL_DEBUG"))
    if debug:
        dbg_s1 = [nc.declare_dram_parameter(f"dbg_s1_{b2}", [P * lay[b2]["R"], D], f32,
                                            isOutput=True) for b2 in range(2)]
        dbg_agg = [nc.declare_dram_parameter(f"dbg_agg{b2}", [P, TB * D], f32,
                                             isOutput=True) for b2 in range(2)]

    s1_d = [nc.dram_tensor(f"s1_{b2}", [P * lay[b2]["R"], D], f32) for b2 in range(2)]

    ctx = ExitStack()
    sb = lambda nm, shape, dt: ctx.enter_context(nc.sbuf_tensor(nm, shape, dt))
    ps = lambda nm, shape: ctx.enter_context(nc.psum_tensor(nm, shape, f32))

    with ctx:
        g_t = [sb(f"g{i}", [P, SLOTS * D], f32) for i in range(GBUFS)]
        idx_t = [sb(f"ix{i}", [P, SLOTS], i32) for i in range(IBUFS)]
        s1sb = [sb(f"s1sb{i}", [P, lay[i]["R"] * D], f32) for i in range(2)]
        agg = [sb(f"agg{i}", [P, TB * D], f32) for i in range(2)]
        cmbg = [sb(f"cmbg{i}", [P, max(1, CM[i] * PMAX) * D], f32) for i in range(2)]
        cmbi_t = [sb(f"cmbit{i}", [P, max(1, CM[i] * PMAX)], i32) for i in range(2)]
        l2i_t = [sb(f"l2it{i}", [P, TB], i32) for i in range(2)]
        recip_t = [sb(f"recipt{i}", [P, TB], f32) for i in range(2)]
        wt_t = sb("wtt", [P, D], f32)
        bias_t = sb("biast", [D, 1], f32)
        id_t = sb("idt", [P, P], f32)
        scaled = [sb(f"scaled{i}", [P, D], f32) for i in range(2)]
        featT = [sb(f"featT{i}", [P, P], f32) for i in range(2)]
        outb = [sb(f"outb{i}", [D, P], f32) for i in range(4)]
        tp1 = [ps(f"tp1_{i}", [D, P]) for i in range(2)]
        mmp = [ps(f"mmp{i}", [D, P]) for i in range(2)]

        with (
            nc.semaphore("hw_sp") as hw_sp,
            nc.semaphore("hw_act") as hw_act,
            nc.semaphore("sw") as sw,
            nc.semaphore("vd") as vd,
            nc.semaphore("ad") as ad,
            nc.semaphore("pe") as pe,
            nc.Block() as block,
        ):
            cnt = dict(hw_sp=0, hw_act=0, sw=0, vd=0, ad=0, pe=0)
            S = dict(sync=[], gpsimd=[], vector=[], scalar=[], tensor=[])
            sem_obj = dict(hw_sp=hw_sp, hw_act=hw_act, sw=sw, vd=vd, ad=ad, pe=pe)

            def emit(eng, fn):
                S[eng].append(fn)

            def wait(eng, sem, val):
                if val > 0:
                    S[eng].append(lambda e, s=sem_obj[sem], v=val: e.wait_ge(s, v))

            def bump(sem, by):
                cnt[sem] += by
                return cnt[sem]

            # ---------- prelude ----------
            def prelude(e):
                e.dma_start(out=wt_t[:], in_=wt_d[:]).then_inc(hw_sp, 16)
                e.dma_start(out=bias_t[:], in_=b_d[:]).then_inc(hw_sp, 16)
                e.dma_start(out=id_t[:], in_=id_d[:]).then_inc(hw_sp, 16)
                e.dma_start(out=recip_t[0][:], in_=recip_d[0]).then_inc(hw_sp, 16)
                e.dma_start(out=recip_t[1][:], in_=recip_d[1]).then_inc(hw_sp, 16)
                e.dma_start(out=l2i_t[0][:], in_=l2i_d[0]).then_inc(hw_sp, 16)
                e.dma_start(out=l2i_t[1][:], in_=l2i_d[1]).then_inc(hw_sp, 16)
            emit("sync", prelude)
            hw_sp_prelude = bump("hw_sp", 7 * 16)

            wait("vector", "hw_sp", hw_sp_prelude)
            def zerocols(e):
                e.memset(s1sb[0][:, 0:D], 0.0)
                e.memset(s1sb[1][:, 0:D], 0.0).then_inc(vd, 1)
            emit("vector", zerocols)
            vd_zero = bump("vd", 1)

            wait("tensor", "hw_sp", hw_sp_prelude)

            # ---------- L1 ----------
            hw_after_idx = {}
            sw_after_chunk = {}
            vd_after_chunk = {}

            def chunk_pieces(beta, ci):
                local = ci - (0 if beta == 0 else NC0)
                return [pc for pc in lay[beta]["pieces"] if pc["chunk"] == local]

            def emit_idx_dma(ci):
                slot = ci % IBUFS
                prev = ci - IBUFS
                if prev >= 0:
                    wait("sync", "sw", sw_after_chunk[prev])
                def f(e, ci=ci, slot=slot):
                    e.dma_start(out=idx_t[slot][:], in_=idxf_d[ci]).then_inc(hw_sp, 16)
                emit("sync", f)
                hw_after_idx[ci] = bump("hw_sp", 16)

            def emit_chunk_gathers(beta, ci):
                slot = ci % IBUFS
                gslot = ci % GBUFS
                wait("gpsimd", "hw_sp", hw_after_idx[ci])
                prevg = ci - GBUFS
                if prevg >= 0:
                    wait("gpsimd", "vd", vd_after_chunk[prevg])
                # HW quirk: the indirect DMA consumes exactly ONE index per
                # partition -> one gather instruction per slot column.
                for pc in chunk_pieces(beta, ci):
                    s0, Wd = pc["col0"], pc["G"] * pc["K"]
                    ph = max(pc["ph"], 2)  # 1-row indirect DMAs unsupported
                    for c in range(s0, s0 + Wd):
                        def f(e, c=c, ph=ph, slot=slot, gslot=gslot):
                            e.indirect_dma_start(
                                out=g_t[gslot][0:ph, c * D:(c + 1) * D],
                                out_offset=None,
                                in_=x_d[:],
                                in_offset=bass.IndirectOffsetOnAxis(
                                    ap=idx_t[slot][0:ph, c:c + 1], axis=0),
                            ).then_inc(sw, 16)
                        emit("gpsimd", f)
                        bump("sw", 16)
                sw_after_chunk[ci] = cnt["sw"]

            def emit_chunk_tree(beta, ci):
                gslot = ci % GBUFS
                wait("vector", "sw", sw_after_chunk[ci])
                pcs = chunk_pieces(beta, ci)
                for pi, pc in enumerate(pcs):
                    K, G, ph, s0, c0 = pc["K"], pc["G"], pc["ph"], pc["col0"], pc["s1c0"]
                    base = g_t[gslot]
                    for (h, w) in _fold_steps(K):
                        def f(e, base=base, ph=ph, G=G, K=K, s0=s0, h=h, w=w):
                            v = base[0:ph, s0 * D:(s0 + G * K) * D].rearrange(
                                "p (g k f) -> p g k f", g=G, f=D)
                            e.tensor_add(out=v[:, :, 0:w - h, :],
                                         in0=v[:, :, 0:w - h, :],
                                         in1=v[:, :, h:w, :])
                        emit("vector", f)
                    is_last = pi == len(pcs) - 1
                    def f(e, base=base, ph=ph, G=G, K=K, s0=s0, c0=c0,
                          beta=beta, is_last=is_last):
                        src = base[0:ph, s0 * D:(s0 + G * K) * D].rearrange(
                            "p (g k f) -> p g k f", g=G, f=D)[:, :, 0, :]
                        dst = s1sb[beta][0:ph, c0 * D:(c0 + G) * D].rearrange(
                            "p (g f) -> p g f", g=G)
                        ins = e.tensor_copy(out=dst, in_=src)
                        if is_last:
                            ins.then_inc(vd, 1)
                    emit("vector", f)
                vd_after_chunk[ci] = bump("vd", 1)

            # ---------- band epilogue ----------
            sw_after_l2 = {}

            def emit_epilogue(beta):
                R = lay[beta]["R"]
                last_ci = (NC0 - 1) if beta == 0 else (NC0 + NC1 - 1)
                cc0 = lay[beta]["cmb_col0"]
                wait("sync", "vd", max(vd_after_chunk[last_ci], vd_zero))
                def f(e, beta=beta, R=R, cc0=cc0):
                    dst = s1_d[beta][:].rearrange("(p r) f -> p r f", r=R)[:, 0:cc0, :]
                    src = s1sb[beta][:, 0:cc0 * D].rearrange("p (r f) -> p r f", f=D)
                    e.dma_start(out=dst, in_=src).then_inc(hw_sp, 16)
                emit("sync", f)
                hw_s1w = bump("hw_sp", 16)
                if CM[beta]:
                    def f(e, beta=beta):
                        e.dma_start(out=cmbi_t[beta][:], in_=cmb_d[beta][:]).then_inc(hw_sp, 16)
                    emit("sync", f)
                    hw_cmbi = bump("hw_sp", 16)
                    wait("gpsimd", "hw_sp", hw_cmbi)
                    for c in range(CM[beta] * PMAX):
                        def f(e, beta=beta, c=c):
                            e.indirect_dma_start(
                                out=cmbg[beta][:, c * D:(c + 1) * D],
                                out_offset=None,
                                in_=s1_d[beta][:],
                                in_offset=bass.IndirectOffsetOnAxis(
                                    ap=cmbi_t[beta][:, c:c + 1], axis=0),
                            ).then_inc(sw, 16)
                        emit("gpsimd", f)
                        bump("sw", 16)
                    sw_cmbg = cnt["sw"]
                    wait("vector", "sw", sw_cmbg)
                    for (h, w) in _fold_steps(PMAX):
                        def f(e, beta=beta, h=h, w=w):
                            v = cmbg[beta][:, 0:CM[beta] * PMAX * D].rearrange(
                                "p (m k f) -> p m k f", m=CM[beta], f=D)
                            e.tensor_add(out=v[:, :, 0:w - h, :],
                                         in0=v[:, :, 0:w - h, :],
                                         in1=v[:, :, h:w, :])
                        emit("vector", f)
                    def f(e, beta=beta, cc0=cc0):
                        src = cmbg[beta][:, 0:CM[beta] * PMAX * D].rearrange(
                            "p (m k f) -> p m k f", m=CM[beta], f=D)[:, :, 0, :]
                        dst = s1sb[beta][:, cc0 * D:(cc0 + CM[beta]) * D].rearrange(
                            "p (m f) -> p m f", f=D)
                        e.tensor_copy(out=dst, in_=src).then_inc(vd, 1)
                    emit("vector", f)
                    vd_cmbf = bump("vd", 1)
                    wait("sync", "vd", vd_cmbf)
                    def f(e, beta=beta, R=R, cc0=cc0):
                        dst = s1_d[beta][:].rearrange("(p r) f -> p r f", r=R)[:, cc0:cc0 + CM[beta], :]
                        src = s1sb[beta][:, cc0 * D:(cc0 + CM[beta]) * D].rearrange(
                            "p (m f) -> p m f", f=D)
                        e.dma_start(out=dst, in_=src).then_inc(hw_sp, 16)
                    emit("sync", f)
                    hw_s1w2 = bump("hw_sp", 16)
                    wait("gpsimd", "hw_sp", hw_s1w2)
                else:
                    wait("gpsimd", "hw_sp", hw_s1w)
                for j in range(TB):
                    def f(e, beta=beta, j=j):
                        e.indirect_dma_start(
                            out=agg[beta][:, j * D:(j + 1) * D],
                            out_offset=None,
                            in_=s1_d[beta][:],
                            in_offset=bass.IndirectOffsetOnAxis(
                                ap=l2i_t[beta][:, j:j + 1], axis=0),
                        ).then_inc(sw, 16)
                    emit("gpsimd", f)
                    bump("sw", 16)
                sw_after_l2[beta] = cnt["sw"]

            # ---------- final phase ----------
            # Per tile: DVE scale -> PE transpose(agg) -> ACT copy to featT
            # top half; xrootT tile DMA'd (host-pretransposed) into featT
            # bottom half; PE matmul W^T @ featT; DVE bias; store the [D, P]
            # result TRANSPOSED to outT (host un-transposes after readback).
            pe_t1, pe_mm = {}, {}
            ad_cpA = {}
            hw_xrt, hw_out = {}, {}
            vd_scale, vd_bias = {}, {}

            def emit_final_tile(beta, j):
                t = beta * TB + j
                s = t % 2
                # DVE: scale by 1/deg
                if j == 0:
                    wait("vector", "sw", sw_after_l2[beta])
                if t - 2 >= 0:
                    wait("vector", "pe", pe_t1[t - 2])   # WAR scaled slot
                def f(e, beta=beta, j=j, s=s):
                    e.tensor_scalar_mul(scaled[s][:], agg[beta][:, j * D:(j + 1) * D],
                                        recip_t[beta][:, j:j + 1]).then_inc(vd, 1)
                emit("vector", f)
                vd_scale[t] = bump("vd", 1)
                # SYNC: xrootT tile load into featT bottom half
                if t - 2 >= 0:
                    wait("sync", "pe", pe_mm[t - 2])     # WAR featT slot
                def f(e, t=t, s=s):
                    e.dma_start(out=featT[s][D:P, :],
                                in_=xrootT_d[:, t * P:(t + 1) * P]).then_inc(hw_sp, 16)
                emit("sync", f)
                hw_xrt[t] = bump("hw_sp", 16)
                # PE: T1
                wait("tensor", "vd", vd_scale[t])
                if t - 2 >= 0:
                    wait("tensor", "ad", ad_cpA[t - 2])  # WAR tp1 psum
                def f(e, s=s):
                    e.matmul(tp1[s][:], lhsT=scaled[s][:], rhs=id_t[:],
                             is_transpose=True, start=True, stop=True).then_inc(pe, 1)
                emit("tensor", f)
                pe_t1[t] = bump("pe", 1)
                # ACT: copyA
                wait("scalar", "pe", pe_t1[t])
                if t - 2 >= 0:
                    wait("scalar", "pe", pe_mm[t - 2])   # WAR featT top half
                def f(e, s=s):
                    e.activation(featT[s][0:D, :], tp1[s][:], COPY).then_inc(ad, 1)
                emit("scalar", f)
                ad_cpA[t] = bump("ad", 1)
                # PE: MM (feat -> out)
                wait("tensor", "ad", ad_cpA[t])
                wait("tensor", "hw_sp", hw_xrt[t])
                if t - 2 >= 0:
                    wait("tensor", "vd", vd_bias[t - 2])  # WAR mmp psum
                def f(e, s=s):
                    e.matmul(mmp[s][:], lhsT=wt_t[:], rhs=featT[s][:],
                             start=True, stop=True).then_inc(pe, 1)
                emit("tensor", f)
                pe_mm[t] = bump("pe", 1)
                # DVE: bias add
                s4 = t % 4
                wait("vector", "pe", pe_mm[t])
                if t - 4 >= 0:
                    wait("vector", "hw_act", hw_out[t - 4])  # WAR outb slot
                def f(e, s=s, s4=s4):
                    e.tensor_add(out=outb[s4][:], in0=mmp[s][:],
                                 in1=bias_t[:].to_broadcast([D, P])).then_inc(vd, 1)
                emit("vector", f)
                vd_bias[t] = bump("vd", 1)
                # ACT: store transposed output tile
                wait("scalar", "vd", vd_bias[t])
                def f(e, t=t, s4=s4):
                    e.dma_start(out=outT_d[:, t * P:(t + 1) * P],
                                in_=outb[s4][:]).then_inc(hw_act, 16)
                emit("scalar", f)
                hw_out[t] = bump("hw_act", 16)

            # ---------- global order ----------
            for ci in range(NC0):
                emit_idx_dma(ci)
                emit_chunk_gathers(0, ci)
                emit_chunk_tree(0, ci)
            pre1 = min(2, NC1)
            for ci in range(NC0, NC0 + pre1):
                emit_idx_dma(ci)
                emit_chunk_gathers(1, ci)
                emit_chunk_tree(1, ci)
            emit_epilogue(0)
            for j in range(TB):
                emit_final_tile(0, j)
            for ci in range(NC0 + pre1, NC0 + NC1):
                emit_idx_dma(ci)
                emit_chunk_gathers(1, ci)
                emit_chunk_tree(1, ci)
            emit_epilogue(1)
            for j in range(TB):
                emit_final_tile(1, j)

            if debug:
                # dump S1 (via DRAM->DRAM copy) and agg tiles
                wait("sync", "sw", cnt["sw"])
                wait("sync", "vd", cnt["vd"])
                def fdbg(e):
                    e.dma_start(out=dbg_s1[0][:], in_=s1_d[0][:]).then_inc(hw_sp, 16)
                    e.dma_start(out=dbg_s1[1][:], in_=s1_d[1][:]).then_inc(hw_sp, 16)
                    e.dma_start(out=dbg_agg[0][:], in_=agg[0][:]).then_inc(hw_sp, 16)
                    e.dma_start(out=dbg_agg[1][:], in_=agg[1][:]).then_inc(hw_sp, 16)
                emit("sync", fdbg)
                bump("hw_sp", 64)

            @block.sync
            def _(eng):
                for f in S["sync"]:
                    f(eng)

            @block.gpsimd
            def _(eng):
                for f in S["gpsimd"]:
                    f(eng)

            @block.vector
            def _(eng):
                for f in S["vector"]:
                    f(eng)

            @block.scalar
            def _(eng):
                for f in S["scalar"]:
                    f(eng)

            @block.tensor
            def _(eng):
                for f in S["tensor"]:
                    f(eng)

    return nc


def kernel(x, row, colptr, W, b):
    global LAST_EXEC_TIME_NS, LAST_MEAN_EXEC_TIME_NS
    from concourse.bass_utils import run_bass_kernel_spmd

    x = np.asarray(x, np.float32)
    row = np.asarray(row, np.int32)
    colptr = np.asarray(colptr, np.int32)
    W = np.asarray(W, np.float32)
    b = np.asarray(b, np.float32)

    plan, in_maps = _plan_and_arrays(x, row, colptr, W, b)
    nc = _build(plan)

    # ensure the axon (neuron) backend is active even if the caller switched
    # jax to cpu (e.g. to run the reference)
    import jax
    if not any(d.platform != "cpu" for d in jax.devices()):
        jax.config.update("jax_platforms", "axon,cpu")
        from jax._src import xla_bridge
        xla_bridge._clear_backends()
        try:
            jax.clear_caches()
        except Exception:
            pass

    trace = bool(os.environ.get("BASS_KERNEL_TRACE"))
    kwargs = {}
    if trace:
        # the NTFF profile hook requires a live axon client connection
        import jax.numpy as jnp
        dev = [d for d in jax.devices() if d.platform != "cpu"][0]
        jax.jit(lambda a: a + 1)(jax.device_put(jnp.zeros((8,)), dev)).block_until_ready()
        kwargs = dict(trace=True, trace_cores=list(range(NCORES)))
    res = run_bass_kernel_spmd(nc, in_maps, list(range(NCORES)), **kwargs)
    LAST_EXEC_TIME_NS = res.exec_time_ns
    LAST_MEAN_EXEC_TIME_NS = res.mean_exec_time_ns
    globals()["LAST_RESULTS"] = res.results
    globals()["LAST_PLAN"] = plan
    globals()["LAST_IN_MAPS"] = in_maps

    out = np.empty((N, D), np.float32)
    bounds = plan["bounds"]
    for c in range(NCORES):
        n0, n1 = int(bounds[c]), int(bounds[c + 1])
        out[n0:n1] = res.results[c]["outT"].T[:n1 - n0]
    return out

